# revision 1
# baseline (speedup 1.0000x reference)
"""Bass/Trainium2 kernel for nn_Net_19602230739296 (NNConv + GRU message passing GNN).

Algorithm (mathematically equivalent to the reference, fp32 everywhere):
  theta[e] = (edge_attr[e] @ nn_w + nn_b).reshape(H, H) is never materialized.
  msg[e]   = sum_c ea'[e,c] * (out[src_e] @ W_c)   with ea' = [edge_attr, 1],
             W_c = nn_w[c].reshape(H,H) for c<4, W_4 = nn_b.reshape(H,H).
  agg^T    = sum_c W_c^T @ (G^T @ Q_c)  per 128-edge tile, where G = out[src]
             (gathered rows) and Q_c[e, slot] = ea'[e,c] * [dst_e == slot-node]
             is a host-precomputed weighted one-hot "scatter" matrix.

Sharding: edges are sorted by destination and packed into tiles of <=128
edges covering <=32 whole destination nodes.  Nodes are renumbered to
(tile*32 + slot).  Each of the 8 cores owns a contiguous range of T tiles
(edge-parallel AND node-parallel at once: a core's edges land only in its own
node range, so no cross-core reduction is needed).  The evolving node
features `out` are replicated via AllGather each iteration; the small
GRU/linear weights are replicated.  All model FLOPs run on device.
"""
import os
import sys

import numpy as np


def _ensure_path():
    for p in ("/opt/trn_rl_repo", os.path.expanduser("~/.axon_site/_ro/trn_rl_repo")):
        if os.path.isdir(p) and p not in sys.path:
            sys.path.insert(0, p)
    try:
        import concourse  # noqa: F401
    except ImportError as e:  # pragma: no cover
        raise ImportError(f"concourse (bass) not importable: {e}")


_ensure_path()

N_NODES, N_EDGES, IN_F, H = 10000, 50000, 32, 64
NCORES = 8
SLOTS = 32            # destination-node slots per tile
EPT = 128             # edge slots per tile
NCH = 5               # edge_attr channels (4) + constant channel for nn_b
T = 56                # tiles per core (fixed so the compiled NEFF is shape-stable)
NTILES = NCORES * T   # 448
NC_COLS = T * SLOTS   # padded nodes per core (1792)
NPAD = NCORES * NC_COLS
CHUNK = 512
# Gather must be chunked: with single_packet=True the whole m2s stream of one
# dma_gather becomes one packet per SDMA engine, and the HW packet ceiling is
# 64 descriptors.  7 tiles -> 896 idxs -> 57 descs/engine.  Rotate chunks
# across the 4 SWDGE queues so descriptor generation runs on 4 Q7 cpu pairs.
GATHER_CHUNKS = 8
N_SWDGE_QUEUES = 4


def _chunks():
    out = []
    c0 = 0
    while c0 < NC_COLS:
        w = min(CHUNK, NC_COLS - c0)
        out.append((c0, w))
        c0 += w
    return out


# ----------------------------------------------------------------------------
# device program
# ----------------------------------------------------------------------------
_NC_CACHE = {}


def _get_nc():
    if "nc" in _NC_CACHE:
        return _NC_CACHE["nc"]
    import concourse.bacc as bacc
    import concourse.mybir as mybir
    import concourse.tile as tile

    dt = mybir.dt
    f32, i16 = dt.float32, dt.int16
    AF = mybir.ActivationFunctionType
    ALU = mybir.AluOpType

    nc = bacc.Bacc(
        "TRN2",
        target_bir_lowering=False,
        debug=False,
        enable_asserts=False,
        num_devices=NCORES,
        num_swdge_queues=N_SWDGE_QUEUES,
    )

    q_in = nc.dram_tensor("q_in", [128, T * NCH * SLOTS], f32, kind="ExternalInput").ap()
    idx_in = nc.dram_tensor("idx_in", [128, T * 8], i16, kind="ExternalInput").ap()
    xt_in = nc.dram_tensor("xt_in", [IN_F, NC_COLS], f32, kind="ExternalInput").ap()
    ws_in = nc.dram_tensor("ws_in", [H, NCH * H], f32, kind="ExternalInput").ap()
    lin0_in = nc.dram_tensor("lin0_in", [IN_F, H], f32, kind="ExternalInput").ap()
    root_in = nc.dram_tensor("root_in", [H, H], f32, kind="ExternalInput").ap()
    wih_in = nc.dram_tensor("wih_in", [H, 3 * H], f32, kind="ExternalInput").ap()
    whh_in = nc.dram_tensor("whh_in", [H, 3 * H], f32, kind="ExternalInput").ap()
    bias_in = nc.dram_tensor("bias_in", [128, 6], f32, kind="ExternalInput").ap()
    ident_in = nc.dram_tensor("ident_in", [128, 128], f32, kind="ExternalInput").ap()
    out_ext = nc.dram_tensor("out_sl", [NC_COLS, H], f32, kind="ExternalOutput").ap()

    chunks = _chunks()

    with tile.TileContext(nc) as tc:
        with tc.tile_pool(name="const", bufs=1) as const, \
             tc.tile_pool(name="work", bufs=1) as work, \
             tc.tile_pool(name="small", bufs=2) as small, \
             tc.tile_pool(name="ps", bufs=4, space="PSUM") as ps, \
             tc.tile_pool(name="aggp", bufs=1, space="PSUM") as aggp, \
             tc.tile_pool(name="dram", bufs=1, space="DRAM") as dram:

            q_sb = const.tile([128, T * NCH * SLOTS], f32, name="q_sb")
            idx_sb = const.tile([128, T * 8], i16, name="idx_sb")
            xt_sb = const.tile([IN_F, NC_COLS], f32, name="xt_sb")
            ws_sb = const.tile([H, NCH * H], f32, name="ws_sb")
            lin0_sb = const.tile([IN_F, H], f32, name="lin0_sb")
            root_sb = const.tile([H, H], f32, name="root_sb")
            wih_sb = const.tile([H, 3 * H], f32, name="wih_sb")
            whh_sb = const.tile([H, 3 * H], f32, name="whh_sb")
            bias_sb = const.tile([128, 6], f32, name="bias_sb")
            ident_sb = const.tile([128, 128], f32, name="ident_sb")

            for sb_t, in_t in (
                (q_sb, q_in), (idx_sb, idx_in), (xt_sb, xt_in), (ws_sb, ws_in),
                (lin0_sb, lin0_in), (root_sb, root_in), (wih_sb, wih_in),
                (whh_sb, whh_in), (bias_sb, bias_in), (ident_sb, ident_in),
            ):
                nc.sync.dma_start(sb_t[:], in_t[:])

            l0b = bias_sb[0:H, 0:1]
            convb = bias_sb[0:H, 1:2]
            br = bias_sb[0:H, 2:3]
            bnih = bias_sb[0:H, 3:4]
            bnhh = bias_sb[0:H, 4:5]
            bz = bias_sb[0:H, 5:6]

            out_a = work.tile([H, NC_COLS], f32, name="out_a")
            out_b = work.tile([H, NC_COLS], f32, name="out_b")
            m_sb = work.tile([H, NC_COLS], f32, name="m_sb")
            row_sb = work.tile([128, (NC_COLS // 128) * H], f32, name="row_sb")
            g_sb = work.tile([128, T * H], f32, name="g_sb")
            s_sb = work.tile([H, T * NCH * SLOTS], f32, name="s_sb")

            agins = [dram.tile([NC_COLS, H], f32, name=f"agin{i}") for i in range(3)]
            agouts = [
                dram.tile([NPAD, H], f32, addr_space="Shared", name=f"agout{i}")
                for i in range(3)
            ]

            def writeback(buf, i):
                # transpose [H, NC_COLS] -> row-major [NC_COLS, H], then AG
                for nt in range(NC_COLS // 128):
                    tp = ps.tile([128, H], f32, tag="w", name=f"tp{i}_{nt}")
                    nc.tensor.transpose(
                        tp[:], buf[:, nt * 128:(nt + 1) * 128], ident_sb[:H, :H]
                    )
                    if nt % 2 == 0:
                        nc.scalar.copy(row_sb[:, nt * H:(nt + 1) * H], tp[:])
                    else:
                        nc.vector.tensor_copy(row_sb[:, nt * H:(nt + 1) * H], tp[:])
                dst = agins[i] if i < 3 else out_ext
                nc.sync.dma_start(
                    dst[:].rearrange("(t p) o -> p t o", p=128),
                    row_sb[:].rearrange("p (t o) -> p t o", o=H),
                )
                if i < 3:
                    nc.gpsimd.collective_compute(
                        "AllGather",
                        mybir.AluOpType.bypass,
                        replica_groups=[list(range(NCORES))],
                        ins=[agins[i][:]],
                        outs=[agouts[i][:]],
                    )

            # ---- iteration 0: out0 = relu(x @ lin0_w + lin0_b), transposed ----
            for k, (c0, w) in enumerate(chunks):
                p0 = ps.tile([64, CHUNK], f32, tag="w", name=f"p0_{k}")
                nc.tensor.matmul(
                    p0[:, :w], lin0_sb[:], xt_sb[:, c0:c0 + w], start=True, stop=True
                )
                nc.scalar.activation(out_a[:, c0:c0 + w], p0[:, :w], AF.Relu, bias=l0b)
            writeback(out_a, 0)

            def edge_phase(it, h):
                src_dram = agouts[it - 1]
                gsz = T // GATHER_CHUNKS
                for gc in range(GATHER_CHUNKS):
                    nc.gpsimd.dma_gather(
                        g_sb[:, gc * gsz * H:(gc + 1) * gsz * H].rearrange(
                            "p (t o) -> p t o", o=H
                        ),
                        src_dram[:],
                        idx_sb[:, gc * gsz * 8:(gc + 1) * gsz * 8],
                        gsz * EPT,
                        gsz * EPT,
                        H,
                        queue_num=gc % N_SWDGE_QUEUES,
                    )
                # matmul1: S_t = G_t^T @ Q_t, 3 tiles per PSUM bank
                t = 0
                while t < T:
                    ntl = min(3, T - t)
                    s_ps = ps.tile([64, CHUNK], f32, tag="w", name=f"sps{it}_{t}")
                    for j in range(ntl):
                        nc.tensor.matmul(
                            s_ps[:, j * 160:(j + 1) * 160],
                            g_sb[:, (t + j) * H:(t + j + 1) * H],
                            q_sb[:, (t + j) * 160:(t + j + 1) * 160],
                            start=True,
                            stop=True,
                        )
                    if (t // 3) % 2 == 0:
                        nc.vector.tensor_copy(
                            s_sb[:, t * 160:(t + ntl) * 160], s_ps[:, :ntl * 160]
                        )
                    else:
                        nc.scalar.copy(
                            s_sb[:, t * 160:(t + ntl) * 160], s_ps[:, :ntl * 160]
                        )
                    t += ntl
                # matmul2: agg^T += W_c^T @ S_c  (16 tiles per matmul, strided rhs)
                agg = aggp.tile([64, 4 * CHUNK], f32, tag="agg", name=f"agg{it}")
                s_re = s_sb[:].rearrange("p (t c s) -> p t c s", c=NCH, s=SLOTS)
                for c in range(NCH):
                    g0 = 0
                    while g0 < T:
                        gn = min(16, T - g0)
                        nc.tensor.matmul(
                            agg[:, g0 * SLOTS:(g0 + gn) * SLOTS],
                            ws_sb[:, c * H:(c + 1) * H],
                            s_re[:, g0:g0 + gn, c, :],
                            start=(c == 0),
                            stop=False,
                        )
                        g0 += gn
                # += out @ root_w  (accumulate into same PSUM region)
                for c0, w in chunks:
                    nc.tensor.matmul(
                        agg[:, c0:c0 + w], root_sb[:], h[:, c0:c0 + w],
                        start=False, stop=True,
                    )
                return agg

            def dense_gru(agg, h, hn, it):
                for k, (c0, w) in enumerate(chunks):
                    nc.scalar.activation(
                        m_sb[:, c0:c0 + w], agg[:, c0:c0 + w], AF.Relu, bias=convb
                    )
                    r_ps = ps.tile([64, CHUNK], f32, tag="w", name=f"r{it}_{k}")
                    nc.tensor.matmul(
                        r_ps[:, :w], wih_sb[:, 0:64], m_sb[:, c0:c0 + w],
                        start=True, stop=False,
                    )
                    nc.tensor.matmul(
                        r_ps[:, :w], whh_sb[:, 0:64], h[:, c0:c0 + w],
                        start=False, stop=True,
                    )
                    r_sb = small.tile([64, CHUNK], f32, tag="rsb", name=f"rs{it}{k}")
                    nc.scalar.activation(r_sb[:, :w], r_ps[:, :w], AF.Sigmoid, bias=br)
                    z_ps = ps.tile([64, CHUNK], f32, tag="w", name=f"z{it}_{k}")
                    nc.tensor.matmul(
                        z_ps[:, :w], wih_sb[:, 64:128], m_sb[:, c0:c0 + w],
                        start=True, stop=False,
                    )
                    nc.tensor.matmul(
                        z_ps[:, :w], whh_sb[:, 64:128], h[:, c0:c0 + w],
                        start=False, stop=True,
                    )
                    z_sb = small.tile([64, CHUNK], f32, tag="zsb", name=f"zs{it}{k}")
                    nc.scalar.activation(z_sb[:, :w], z_ps[:, :w], AF.Sigmoid, bias=bz)
                    n1 = ps.tile([64, CHUNK], f32, tag="w", name=f"n1{it}_{k}")
                    nc.tensor.matmul(
                        n1[:, :w], wih_sb[:, 128:192], m_sb[:, c0:c0 + w],
                        start=True, stop=True,
                    )
                    n2 = ps.tile([64, CHUNK], f32, tag="w", name=f"n2{it}_{k}")
                    nc.tensor.matmul(
                        n2[:, :w], whh_sb[:, 128:192], h[:, c0:c0 + w],
                        start=True, stop=True,
                    )
                    # tmp = (n2 + b_hh_n) * r
                    tmp = small.tile([64, CHUNK], f32, tag="tmp", name=f"tmp{it}{k}")
                    nc.vector.scalar_tensor_tensor(
                        tmp[:, :w], n2[:, :w], bnhh, r_sb[:, :w], ALU.add, ALU.mult
                    )
                    pre = small.tile([64, CHUNK], f32, tag="pre", name=f"pre{it}{k}")
                    nc.vector.tensor_add(pre[:, :w], n1[:, :w], tmp[:, :w])
                    nsb = small.tile([64, CHUNK], f32, tag="nsb", name=f"nsb{it}{k}")
                    nc.scalar.activation(nsb[:, :w], pre[:, :w], AF.Tanh, bias=bnih)
                    # h' = n + z * (h - n)
                    dd = small.tile([64, CHUNK], f32, tag="dd", name=f"dd{it}{k}")
                    nc.vector.tensor_sub(dd[:, :w], h[:, c0:c0 + w], nsb[:, :w])
                    t4 = small.tile([64, CHUNK], f32, tag="t4", name=f"t4{it}{k}")
                    nc.vector.tensor_mul(t4[:, :w], z_sb[:, :w], dd[:, :w])
                    nc.vector.tensor_add(hn[:, c0:c0 + w], nsb[:, :w], t4[:, :w])

            h, hn = out_a, out_b
            for it in (1, 2, 3):
                agg = edge_phase(it, h)
                dense_gru(agg, h, hn, it)
                writeback(hn, it)
                h, hn = hn, h

    nc.compile()
    _NC_CACHE["nc"] = nc
    return nc


# ----------------------------------------------------------------------------
# host-side graph preprocessing (pure data layout, no model FLOPs)
# ----------------------------------------------------------------------------
def _pack(edge_index, edge_attr):
    src = np.asarray(edge_index[0]).astype(np.int64)
    dst = np.asarray(edge_index[1]).astype(np.int64)
    ea = np.asarray(edge_attr, np.float32)
    order = np.argsort(dst, kind="stable")
    ssrc, sea = src[order], ea[order]
    deg = np.bincount(dst, minlength=N_NODES)
    starts = np.zeros(N_NODES + 1, np.int64)
    starts[1:] = np.cumsum(deg)
    uniq = np.flatnonzero(deg)
    zs = np.flatnonzero(deg == 0)
    node_seq = np.concatenate([uniq, zs])

    tiles_nodes = [[]]
    ce = 0
    for nd in node_seq:
        d = int(deg[nd])
        assert d <= EPT, f"node degree {d} exceeds edge tile capacity"
        if len(tiles_nodes[-1]) >= SLOTS or ce + d > EPT:
            tiles_nodes.append([])
            ce = 0
        tiles_nodes[-1].append(int(nd))
        ce += d
    assert len(tiles_nodes) <= NTILES, f"need {len(tiles_nodes)} tiles > {NTILES}"

    perm = np.empty(N_NODES, np.int64)
    for t, nodes in enumerate(tiles_nodes):
        for j, nd in enumerate(nodes):
            perm[nd] = t * SLOTS + j

    q = np.zeros((NTILES, EPT, NCH, SLOTS), np.float32)
    srcslot = np.zeros((NTILES, EPT), np.int16)
    for t, nodes in enumerate(tiles_nodes):
        e = 0
        for j, nd in enumerate(nodes):
            s0, s1 = int(starts[nd]), int(starts[nd + 1])
            ne = s1 - s0
            if ne:
                q[t, e:e + ne, 0:4, j] = sea[s0:s1]
                q[t, e:e + ne, 4, j] = 1.0
                srcslot[t, e:e + ne] = perm[ssrc[s0:s1]].astype(np.int16)
                e += ne

    qs, idxs = [], []
    i_arange = np.arange(T * EPT)
    for k in range(NCORES):
        qt = q[k * T:(k + 1) * T]
        qs.append(
            np.ascontiguousarray(qt.transpose(1, 0, 2, 3)).reshape(
                128, T * NCH * SLOTS
            )
        )
        flat = srcslot[k * T:(k + 1) * T].reshape(-1)
        ia = np.zeros((128, T * 8), np.int16)
        # the index list is read per 16-partition group by each of the 8
        # GPSIMD cores on HW -> replicate it into every group
        for g in range(8):
            ia[g * 16 + i_arange % 16, i_arange // 16] = flat
        idxs.append(ia)
    return qs, idxs, perm


def _prep_inputs(inputs):
    x = np.asarray(inputs["x"], np.float32)
    qs, idxs, perm = _pack(inputs["edge_index"], inputs["edge_attr"])

    x_pad = np.zeros((NPAD, IN_F), np.float32)
    x_pad[perm] = x
    xts = [
        np.ascontiguousarray(x_pad[k * NC_COLS:(k + 1) * NC_COLS].T)
        for k in range(NCORES)
    ]

    nw = np.asarray(inputs["nn_w"], np.float32)
    ws = np.zeros((H, NCH * H), np.float32)
    for c in range(4):
        ws[:, c * H:(c + 1) * H] = nw[c].reshape(H, H)
    ws[:, 4 * H:5 * H] = np.asarray(inputs["nn_b"], np.float32).reshape(H, H)

    lin0_w = np.ascontiguousarray(np.asarray(inputs["lin0_w"], np.float32))
    root_w = np.ascontiguousarray(np.asarray(inputs["root_w"], np.float32))
    wih_t = np.ascontiguousarray(np.asarray(inputs["gru_w_ih"], np.float32).T)
    whh_t = np.ascontiguousarray(np.asarray(inputs["gru_w_hh"], np.float32).T)
    b_ih = np.asarray(inputs["gru_b_ih"], np.float32)
    b_hh = np.asarray(inputs["gru_b_hh"], np.float32)

    bias_pack = np.zeros((128, 6), np.float32)
    bias_pack[0:H, 0] = np.asarray(inputs["lin0_b"], np.float32)
    bias_pack[0:H, 1] = np.asarray(inputs["conv_b"], np.float32)
    bias_pack[0:H, 2] = (b_ih + b_hh)[0:64]
    bias_pack[0:H, 3] = b_ih[128:192]
    bias_pack[0:H, 4] = b_hh[128:192]
    bias_pack[0:H, 5] = (b_ih + b_hh)[64:128]
    ident = np.eye(128, dtype=np.float32)

    in_maps = []
    for k in range(NCORES):
        in_maps.append(
            {
                "q_in": qs[k],
                "idx_in": idxs[k],
                "xt_in": xts[k],
                "ws_in": ws,
                "lin0_in": lin0_w,
                "root_in": root_w,
                "wih_in": wih_t,
                "whh_in": whh_t,
                "bias_in": bias_pack,
                "ident_in": ident,
            }
        )
    return in_maps, perm


def _assemble(results, perm):
    full = np.concatenate([results[k]["out_sl"] for k in range(NCORES)], axis=0)
    return np.ascontiguousarray(full[perm]).astype(np.float32)


def kernel(**inputs) -> np.ndarray:
    in_maps, perm = _prep_inputs(inputs)
    nc = _get_nc()
    if os.environ.get("BASS_KERNEL_SIM"):
        results = _run_sim(nc, in_maps)
    else:
        from concourse import bass_utils

        res = bass_utils.run_bass_kernel_spmd(
            nc, in_maps, core_ids=list(range(NCORES))
        )
        results = res.results
    return _assemble(results, perm)


def _run_sim(nc, in_maps):
    from concourse.bass_interp import MultiCoreSim

    sim = MultiCoreSim(nc, num_cores=NCORES, trace=False)
    for k, core in sim.cores.items():
        for name, arr in in_maps[k].items():
            core.tensor(name)[:] = arr
    sim.simulate(check_with_hw=False)
    out = []
    for k in range(NCORES):
        out.append({"out_sl": np.array(sim.cores[k].tensor("out_sl"))})
    return out


if __name__ == "__main__":
    rng = np.random.default_rng(0)
    demo = {
        "x": rng.standard_normal((N_NODES, IN_F), dtype=np.float32),
        "edge_index": rng.integers(0, N_NODES, (2, N_EDGES)).astype(np.int32),
        "edge_attr": rng.random((N_EDGES, 4), dtype=np.float32),
        "lin0_w": rng.standard_normal((IN_F, H), dtype=np.float32) * 0.1,
        "lin0_b": np.zeros(H, np.float32),
        "nn_w": rng.standard_normal((4, H * H), dtype=np.float32) * 0.05,
        "nn_b": np.zeros(H * H, np.float32),
        "root_w": rng.standard_normal((H, H), dtype=np.float32) * 0.1,
        "conv_b": np.zeros(H, np.float32),
        "gru_w_ih": rng.standard_normal((3 * H, H), dtype=np.float32) * 0.1,
        "gru_w_hh": rng.standard_normal((3 * H, H), dtype=np.float32) * 0.1,
        "gru_b_ih": np.zeros(3 * H, np.float32),
        "gru_b_hh": np.zeros(3 * H, np.float32),
    }
    out = kernel(**demo)
    print("kernel output", out.shape, out.dtype, float(np.abs(out).mean()))



# revision 4
# speedup vs baseline: 1.4785x; 1.4785x over previous
"""Bass/Trainium2 kernel for nn_Net_19602230739296 (NNConv + GRU message passing GNN).

Algorithm (mathematically equivalent to the reference, fp32 everywhere):
  theta[e] = (edge_attr[e] @ nn_w + nn_b).reshape(H, H) is never materialized.
  msg[e]   = sum_c ea'[e,c] * (out[src_e] @ W_c)   with ea' = [edge_attr, 1],
             W_c = nn_w[c].reshape(H,H) for c<4, W_4 = nn_b.reshape(H,H).
  agg^T    = sum_c W_c^T @ (G^T @ Q_c)  per 128-edge tile, where G = out[src]
             (gathered rows) and Q_c[e, slot] = ea'[e,c] * [dst_e == slot-node]
             is a host-precomputed weighted one-hot "scatter" matrix.

Sharding: edges are sorted by destination and packed into tiles of <=128
edges covering <=32 whole destination nodes.  Nodes are renumbered to
(tile*32 + slot).  Each of the 8 cores owns a contiguous range of T tiles
(edge-parallel AND node-parallel at once: a core's edges land only in its own
node range, so no cross-core reduction is needed).  The evolving node
features `out` are replicated via AllGather each iteration; the small
GRU/linear weights are replicated.  All model FLOPs run on device.
"""
import os
import sys

import numpy as np


def _ensure_path():
    for p in ("/opt/trn_rl_repo", os.path.expanduser("~/.axon_site/_ro/trn_rl_repo")):
        if os.path.isdir(p) and p not in sys.path:
            sys.path.insert(0, p)
    try:
        import concourse  # noqa: F401
    except ImportError as e:  # pragma: no cover
        raise ImportError(f"concourse (bass) not importable: {e}")


_ensure_path()

N_NODES, N_EDGES, IN_F, H = 10000, 50000, 32, 64
NCORES = 8
SLOTS = 32            # destination-node slots per tile
EPT = 128             # edge slots per tile
NCH = 5               # edge_attr channels (4) + constant channel for nn_b
T = 56                # tiles per core (fixed so the compiled NEFF is shape-stable)
NTILES = NCORES * T   # 448
NC_COLS = T * SLOTS   # padded nodes per core (1792)
NPAD = NCORES * NC_COLS
CHUNK = 512
# Gather must be chunked: with single_packet=True the whole m2s stream of one
# dma_gather becomes one packet per SDMA engine, and the HW packet ceiling is
# 64 descriptors.  7 tiles -> 896 idxs -> 57 descs/engine.  Rotate chunks
# across the 4 SWDGE queues so descriptor generation runs on 4 Q7 cpu pairs.
GATHER_CHUNKS = 8
N_SWDGE_QUEUES = 4


def _chunks():
    out = []
    c0 = 0
    while c0 < NC_COLS:
        w = min(CHUNK, NC_COLS - c0)
        out.append((c0, w))
        c0 += w
    return out


# ----------------------------------------------------------------------------
# device program
# ----------------------------------------------------------------------------
_NC_CACHE = {}


def _get_nc():
    if "nc" in _NC_CACHE:
        return _NC_CACHE["nc"]
    import concourse.bacc as bacc
    import concourse.mybir as mybir
    import concourse.tile as tile

    dt = mybir.dt
    f32, i16 = dt.float32, dt.int16
    AF = mybir.ActivationFunctionType
    ALU = mybir.AluOpType

    nc = bacc.Bacc(
        "TRN2",
        target_bir_lowering=False,
        debug=False,
        enable_asserts=False,
        num_devices=NCORES,
        num_swdge_queues=N_SWDGE_QUEUES,
    )

    q_in = nc.dram_tensor("q_in", [128, T * NCH * SLOTS], f32, kind="ExternalInput").ap()
    idx_in = nc.dram_tensor("idx_in", [128, T * 8], i16, kind="ExternalInput").ap()
    xt_in = nc.dram_tensor("xt_in", [IN_F, NC_COLS], f32, kind="ExternalInput").ap()
    ws_in = nc.dram_tensor("ws_in", [H, NCH * H], f32, kind="ExternalInput").ap()
    lin0_in = nc.dram_tensor("lin0_in", [IN_F, H], f32, kind="ExternalInput").ap()
    root_in = nc.dram_tensor("root_in", [H, H], f32, kind="ExternalInput").ap()
    wih_in = nc.dram_tensor("wih_in", [H, 3 * H], f32, kind="ExternalInput").ap()
    whh_in = nc.dram_tensor("whh_in", [H, 3 * H], f32, kind="ExternalInput").ap()
    bias_in = nc.dram_tensor("bias_in", [128, 6], f32, kind="ExternalInput").ap()
    ident_in = nc.dram_tensor("ident_in", [128, 128], f32, kind="ExternalInput").ap()
    out_ext = nc.dram_tensor("out_sl", [NC_COLS, H], f32, kind="ExternalOutput").ap()

    chunks = _chunks()

    with tile.TileContext(nc) as tc:
        with tc.tile_pool(name="const", bufs=1) as const, \
             tc.tile_pool(name="work", bufs=1) as work, \
             tc.tile_pool(name="small", bufs=2) as small, \
             tc.tile_pool(name="ps", bufs=4, space="PSUM") as ps, \
             tc.tile_pool(name="aggp", bufs=1, space="PSUM") as aggp, \
             tc.tile_pool(name="dram", bufs=1, space="DRAM") as dram:

            q_sb = const.tile([128, T * NCH * SLOTS], f32, name="q_sb")
            idx_sb = const.tile([128, T * 8], i16, name="idx_sb")
            xt_sb = const.tile([IN_F, NC_COLS], f32, name="xt_sb")
            ws_sb = const.tile([H, NCH * H], f32, name="ws_sb")
            lin0_sb = const.tile([IN_F, H], f32, name="lin0_sb")
            root_sb = const.tile([H, H], f32, name="root_sb")
            wih_sb = const.tile([H, 3 * H], f32, name="wih_sb")
            whh_sb = const.tile([H, 3 * H], f32, name="whh_sb")
            bias_sb = const.tile([128, 6], f32, name="bias_sb")
            ident_sb = const.tile([128, 128], f32, name="ident_sb")

            for sb_t, in_t in (
                (xt_sb, xt_in), (lin0_sb, lin0_in), (bias_sb, bias_in),
                (ident_sb, ident_in), (idx_sb, idx_in), (ws_sb, ws_in),
                (root_sb, root_in), (wih_sb, wih_in), (whh_sb, whh_in),
                (q_sb, q_in),
            ):
                nc.sync.dma_start(sb_t[:], in_t[:])

            l0b = bias_sb[0:H, 0:1]
            convb = bias_sb[0:H, 1:2]
            br = bias_sb[0:H, 2:3]
            bnih = bias_sb[0:H, 3:4]
            bnhh = bias_sb[0:H, 4:5]
            bz = bias_sb[0:H, 5:6]

            out_a = work.tile([H, NC_COLS], f32, name="out_a")
            out_b = work.tile([H, NC_COLS], f32, name="out_b")
            m_sb = work.tile([H, NC_COLS], f32, name="m_sb")
            row_sb = work.tile([128, (NC_COLS // 128) * H], f32, name="row_sb")
            g_sb = work.tile([128, T * H], f32, name="g_sb")
            s_sb = work.tile([H, T * NCH * SLOTS], f32, name="s_sb")

            agins = [dram.tile([NC_COLS, H], f32, name=f"agin{i}") for i in range(3)]
            agouts = [
                dram.tile([NPAD, H], f32, addr_space="Shared", name=f"agout{i}")
                for i in range(3)
            ]

            def writeback(buf, i):
                # transpose [H, NC_COLS] -> row-major [NC_COLS, H], then AG
                for nt in range(NC_COLS // 128):
                    tp = ps.tile([128, H], f32, tag="w", name=f"tp{i}_{nt}")
                    nc.tensor.transpose(
                        tp[:], buf[:, nt * 128:(nt + 1) * 128], ident_sb[:H, :H]
                    )
                    if nt % 2 == 0:
                        nc.scalar.copy(row_sb[:, nt * H:(nt + 1) * H], tp[:])
                    else:
                        nc.vector.tensor_copy(row_sb[:, nt * H:(nt + 1) * H], tp[:])
                dst = agins[i] if i < 3 else out_ext
                nc.sync.dma_start(
                    dst[:].rearrange("(t p) o -> p t o", p=128),
                    row_sb[:].rearrange("p (t o) -> p t o", o=H),
                )
                if i < 3:
                    nc.gpsimd.collective_compute(
                        "AllGather",
                        mybir.AluOpType.bypass,
                        replica_groups=[list(range(NCORES))],
                        ins=[agins[i][:]],
                        outs=[agouts[i][:]],
                    )

            # ---- iteration 0: out0 = relu(x @ lin0_w + lin0_b), transposed ----
            for k, (c0, w) in enumerate(chunks):
                p0 = ps.tile([64, CHUNK], f32, tag="w", name=f"p0_{k}")
                nc.tensor.matmul(
                    p0[:, :w], lin0_sb[:], xt_sb[:, c0:c0 + w], start=True, stop=True
                )
                nc.scalar.activation(out_a[:, c0:c0 + w], p0[:, :w], AF.Relu, bias=l0b)
            writeback(out_a, 0)

            def edge_phase(it, h):
                src_dram = agouts[it - 1]
                gsz = T // GATHER_CHUNKS
                for gc in range(GATHER_CHUNKS):
                    nc.gpsimd.dma_gather(
                        g_sb[:, gc * gsz * H:(gc + 1) * gsz * H].rearrange(
                            "p (t o) -> p t o", o=H
                        ),
                        src_dram[:],
                        idx_sb[:, gc * gsz * 8:(gc + 1) * gsz * 8],
                        gsz * EPT,
                        gsz * EPT,
                        H,
                        queue_num=gc % N_SWDGE_QUEUES,
                    )
                # matmul1: S_t = G_t^T @ Q_t, 3 tiles per PSUM bank
                t = 0
                while t < T:
                    ntl = min(3, T - t)
                    s_ps = ps.tile([64, CHUNK], f32, tag="w", name=f"sps{it}_{t}")
                    for j in range(ntl):
                        nc.tensor.matmul(
                            s_ps[:, j * 160:(j + 1) * 160],
                            g_sb[:, (t + j) * H:(t + j + 1) * H],
                            q_sb[:, (t + j) * 160:(t + j + 1) * 160],
                            start=True,
                            stop=True,
                        )
                    if (t // 3) % 2 == 0:
                        nc.vector.tensor_copy(
                            s_sb[:, t * 160:(t + ntl) * 160], s_ps[:, :ntl * 160]
                        )
                    else:
                        nc.scalar.copy(
                            s_sb[:, t * 160:(t + ntl) * 160], s_ps[:, :ntl * 160]
                        )
                    t += ntl
                # matmul2: agg^T += W_c^T @ S_c  (16 tiles per matmul, strided rhs)
                agg = aggp.tile([64, 4 * CHUNK], f32, tag="agg", name=f"agg{it}")
                s_re = s_sb[:].rearrange("p (t c s) -> p t c s", c=NCH, s=SLOTS)
                for c in range(NCH):
                    g0 = 0
                    while g0 < T:
                        gn = min(16, T - g0)
                        nc.tensor.matmul(
                            agg[:, g0 * SLOTS:(g0 + gn) * SLOTS],
                            ws_sb[:, c * H:(c + 1) * H],
                            s_re[:, g0:g0 + gn, c, :],
                            start=(c == 0),
                            stop=False,
                        )
                        g0 += gn
                # += out @ root_w  (accumulate into same PSUM region)
                for c0, w in chunks:
                    nc.tensor.matmul(
                        agg[:, c0:c0 + w], root_sb[:], h[:, c0:c0 + w],
                        start=False, stop=True,
                    )
                return agg

            def dense_gru(agg, h, hn, it):
                for k, (c0, w) in enumerate(chunks):
                    nc.scalar.activation(
                        m_sb[:, c0:c0 + w], agg[:, c0:c0 + w], AF.Relu, bias=convb
                    )
                    r_ps = ps.tile([64, CHUNK], f32, tag="w", name=f"r{it}_{k}")
                    nc.tensor.matmul(
                        r_ps[:, :w], wih_sb[:, 0:64], m_sb[:, c0:c0 + w],
                        start=True, stop=False,
                    )
                    nc.tensor.matmul(
                        r_ps[:, :w], whh_sb[:, 0:64], h[:, c0:c0 + w],
                        start=False, stop=True,
                    )
                    r_sb = small.tile([64, CHUNK], f32, tag="rsb", name=f"rs{it}{k}")
                    nc.scalar.activation(r_sb[:, :w], r_ps[:, :w], AF.Sigmoid, bias=br)
                    z_ps = ps.tile([64, CHUNK], f32, tag="w", name=f"z{it}_{k}")
                    nc.tensor.matmul(
                        z_ps[:, :w], wih_sb[:, 64:128], m_sb[:, c0:c0 + w],
                        start=True, stop=False,
                    )
                    nc.tensor.matmul(
                        z_ps[:, :w], whh_sb[:, 64:128], h[:, c0:c0 + w],
                        start=False, stop=True,
                    )
                    z_sb = small.tile([64, CHUNK], f32, tag="zsb", name=f"zs{it}{k}")
                    nc.scalar.activation(z_sb[:, :w], z_ps[:, :w], AF.Sigmoid, bias=bz)
                    n1 = ps.tile([64, CHUNK], f32, tag="w", name=f"n1{it}_{k}")
                    nc.tensor.matmul(
                        n1[:, :w], wih_sb[:, 128:192], m_sb[:, c0:c0 + w],
                        start=True, stop=True,
                    )
                    n2 = ps.tile([64, CHUNK], f32, tag="w", name=f"n2{it}_{k}")
                    nc.tensor.matmul(
                        n2[:, :w], whh_sb[:, 128:192], h[:, c0:c0 + w],
                        start=True, stop=True,
                    )
                    # tmp = (n2 + b_hh_n) * r
                    tmp = small.tile([64, CHUNK], f32, tag="tmp", name=f"tmp{it}{k}")
                    nc.vector.scalar_tensor_tensor(
                        tmp[:, :w], n2[:, :w], bnhh, r_sb[:, :w], ALU.add, ALU.mult
                    )
                    pre = small.tile([64, CHUNK], f32, tag="pre", name=f"pre{it}{k}")
                    nc.vector.tensor_add(pre[:, :w], n1[:, :w], tmp[:, :w])
                    nsb = small.tile([64, CHUNK], f32, tag="nsb", name=f"nsb{it}{k}")
                    nc.scalar.activation(nsb[:, :w], pre[:, :w], AF.Tanh, bias=bnih)
                    # h' = n + z * (h - n)
                    dd = small.tile([64, CHUNK], f32, tag="dd", name=f"dd{it}{k}")
                    nc.vector.tensor_sub(dd[:, :w], h[:, c0:c0 + w], nsb[:, :w])
                    t4 = small.tile([64, CHUNK], f32, tag="t4", name=f"t4{it}{k}")
                    nc.vector.tensor_mul(t4[:, :w], z_sb[:, :w], dd[:, :w])
                    nc.vector.tensor_add(hn[:, c0:c0 + w], nsb[:, :w], t4[:, :w])

            h, hn = out_a, out_b
            for it in (1, 2, 3):
                agg = edge_phase(it, h)
                dense_gru(agg, h, hn, it)
                writeback(hn, it)
                h, hn = hn, h

    nc.compile()
    _NC_CACHE["nc"] = nc
    return nc


# ----------------------------------------------------------------------------
# host-side graph preprocessing (pure data layout, no model FLOPs)
# ----------------------------------------------------------------------------
def _pack(edge_index, edge_attr):
    src = np.asarray(edge_index[0]).astype(np.int64)
    dst = np.asarray(edge_index[1]).astype(np.int64)
    ea = np.asarray(edge_attr, np.float32)
    order = np.argsort(dst, kind="stable")
    ssrc, sea = src[order], ea[order]
    deg = np.bincount(dst, minlength=N_NODES)
    starts = np.zeros(N_NODES + 1, np.int64)
    starts[1:] = np.cumsum(deg)
    uniq = np.flatnonzero(deg)
    zs = np.flatnonzero(deg == 0)
    node_seq = np.concatenate([uniq, zs])

    raw_tiles = [[]]
    ce = 0
    for nd in node_seq:
        d = int(deg[nd])
        assert d <= EPT, f"node degree {d} exceeds edge tile capacity"
        if len(raw_tiles[-1]) >= SLOTS or ce + d > EPT:
            raw_tiles.append([])
            ce = 0
        raw_tiles[-1].append(int(nd))
        ce += d
    assert len(raw_tiles) <= NTILES, f"need {len(raw_tiles)} tiles > {NTILES}"
    # Distribute real tiles round-robin across the 8 cores so every core gets
    # an equal share of real edges (a contiguous split leaves the last core
    # nearly all padding, which skews its runtime and stalls the collectives).
    tiles_nodes = [[] for _ in range(NTILES)]
    for i, nodes in enumerate(raw_tiles):
        core, j = i % NCORES, i // NCORES
        tiles_nodes[core * T + j] = nodes

    perm = np.empty(N_NODES, np.int64)
    for t, nodes in enumerate(tiles_nodes):
        for j, nd in enumerate(nodes):
            perm[nd] = t * SLOTS + j

    q = np.zeros((NTILES, EPT, NCH, SLOTS), np.float32)
    # Padding gather slots must NOT all point at row 0: thousands of reads of
    # one 256B row serialize on a single HBM bank.  Interspersed padding gets
    # spread distinct rows in the core's own slice; trailing padding (after
    # the final real edge of a gather chunk) is -1, which SWDGE skips.
    srcslot = np.full((NTILES, EPT), -1, np.int16)
    nreal = np.zeros(NTILES, np.int64)
    for t, nodes in enumerate(tiles_nodes):
        e = 0
        for j, nd in enumerate(nodes):
            s0, s1 = int(starts[nd]), int(starts[nd + 1])
            ne = s1 - s0
            if ne:
                q[t, e:e + ne, 0:4, j] = sea[s0:s1]
                q[t, e:e + ne, 4, j] = 1.0
                srcslot[t, e:e + ne] = perm[ssrc[s0:s1]].astype(np.int16)
                e += ne
        nreal[t] = e
    for k in range(NCORES):
        base = k * NC_COLS
        block = srcslot[k * T:(k + 1) * T].reshape(-1)
        holes = np.flatnonzero(block < 0)
        block[holes] = base + np.arange(holes.size) % NC_COLS
        srcslot[k * T:(k + 1) * T] = block.reshape(T, EPT)

    qs, idxs = [], []
    i_arange = np.arange(T * EPT)
    for k in range(NCORES):
        qt = q[k * T:(k + 1) * T]
        qs.append(
            np.ascontiguousarray(qt.transpose(1, 0, 2, 3)).reshape(
                128, T * NCH * SLOTS
            )
        )
        flat = srcslot[k * T:(k + 1) * T].reshape(-1)
        ia = np.zeros((128, T * 8), np.int16)
        # the index list is read per 16-partition group by each of the 8
        # GPSIMD cores on HW -> replicate it into every group
        for g in range(8):
            ia[g * 16 + i_arange % 16, i_arange // 16] = flat
        idxs.append(ia)
    return qs, idxs, perm


def _prep_inputs(inputs):
    x = np.asarray(inputs["x"], np.float32)
    qs, idxs, perm = _pack(inputs["edge_index"], inputs["edge_attr"])

    x_pad = np.zeros((NPAD, IN_F), np.float32)
    x_pad[perm] = x
    xts = [
        np.ascontiguousarray(x_pad[k * NC_COLS:(k + 1) * NC_COLS].T)
        for k in range(NCORES)
    ]

    nw = np.asarray(inputs["nn_w"], np.float32)
    ws = np.zeros((H, NCH * H), np.float32)
    for c in range(4):
        ws[:, c * H:(c + 1) * H] = nw[c].reshape(H, H)
    ws[:, 4 * H:5 * H] = np.asarray(inputs["nn_b"], np.float32).reshape(H, H)

    lin0_w = np.ascontiguousarray(np.asarray(inputs["lin0_w"], np.float32))
    root_w = np.ascontiguousarray(np.asarray(inputs["root_w"], np.float32))
    wih_t = np.ascontiguousarray(np.asarray(inputs["gru_w_ih"], np.float32).T)
    whh_t = np.ascontiguousarray(np.asarray(inputs["gru_w_hh"], np.float32).T)
    b_ih = np.asarray(inputs["gru_b_ih"], np.float32)
    b_hh = np.asarray(inputs["gru_b_hh"], np.float32)

    bias_pack = np.zeros((128, 6), np.float32)
    bias_pack[0:H, 0] = np.asarray(inputs["lin0_b"], np.float32)
    bias_pack[0:H, 1] = np.asarray(inputs["conv_b"], np.float32)
    bias_pack[0:H, 2] = (b_ih + b_hh)[0:64]
    bias_pack[0:H, 3] = b_ih[128:192]
    bias_pack[0:H, 4] = b_hh[128:192]
    bias_pack[0:H, 5] = (b_ih + b_hh)[64:128]
    ident = np.eye(128, dtype=np.float32)

    in_maps = []
    for k in range(NCORES):
        in_maps.append(
            {
                "q_in": qs[k],
                "idx_in": idxs[k],
                "xt_in": xts[k],
                "ws_in": ws,
                "lin0_in": lin0_w,
                "root_in": root_w,
                "wih_in": wih_t,
                "whh_in": whh_t,
                "bias_in": bias_pack,
                "ident_in": ident,
            }
        )
    return in_maps, perm


def _assemble(results, perm):
    full = np.concatenate([results[k]["out_sl"] for k in range(NCORES)], axis=0)
    return np.ascontiguousarray(full[perm]).astype(np.float32)


def kernel(**inputs) -> np.ndarray:
    in_maps, perm = _prep_inputs(inputs)
    nc = _get_nc()
    if os.environ.get("BASS_KERNEL_SIM"):
        results = _run_sim(nc, in_maps)
    else:
        from concourse import bass_utils

        res = bass_utils.run_bass_kernel_spmd(
            nc, in_maps, core_ids=list(range(NCORES))
        )
        results = res.results
    return _assemble(results, perm)


def _run_sim(nc, in_maps):
    from concourse.bass_interp import MultiCoreSim

    sim = MultiCoreSim(nc, num_cores=NCORES, trace=False)
    for k, core in sim.cores.items():
        for name, arr in in_maps[k].items():
            core.tensor(name)[:] = arr
    sim.simulate(check_with_hw=False)
    out = []
    for k in range(NCORES):
        out.append({"out_sl": np.array(sim.cores[k].tensor("out_sl"))})
    return out


if __name__ == "__main__":
    rng = np.random.default_rng(0)
    demo = {
        "x": rng.standard_normal((N_NODES, IN_F), dtype=np.float32),
        "edge_index": rng.integers(0, N_NODES, (2, N_EDGES)).astype(np.int32),
        "edge_attr": rng.random((N_EDGES, 4), dtype=np.float32),
        "lin0_w": rng.standard_normal((IN_F, H), dtype=np.float32) * 0.1,
        "lin0_b": np.zeros(H, np.float32),
        "nn_w": rng.standard_normal((4, H * H), dtype=np.float32) * 0.05,
        "nn_b": np.zeros(H * H, np.float32),
        "root_w": rng.standard_normal((H, H), dtype=np.float32) * 0.1,
        "conv_b": np.zeros(H, np.float32),
        "gru_w_ih": rng.standard_normal((3 * H, H), dtype=np.float32) * 0.1,
        "gru_w_hh": rng.standard_normal((3 * H, H), dtype=np.float32) * 0.1,
        "gru_b_ih": np.zeros(3 * H, np.float32),
        "gru_b_hh": np.zeros(3 * H, np.float32),
    }
    out = kernel(**demo)
    print("kernel output", out.shape, out.dtype, float(np.abs(out).mean()))



# revision 11
# speedup vs baseline: 1.6283x; 1.1013x over previous
"""Bass/Trainium2 kernel for nn_Net_19602230739296 (NNConv + GRU message passing GNN).

Algorithm (mathematically equivalent to the reference):
  theta[e] = (edge_attr[e] @ nn_w + nn_b).reshape(H, H) is never materialized.
  msg[e]   = sum_c ea'[e,c] * (out[src_e] @ W_c)   with ea' = [edge_attr, 1],
             W_c = nn_w[c].reshape(H,H) for c<4, W_4 = nn_b.reshape(H,H).
  agg^T    = sum_c W_c^T @ (G^T @ Q_c)  per 128-edge tile, where G = out[src]
             (gathered rows) and Q_c[e, slot] = ea'[e,c] * [dst_e == slot-node]
             is a host-precomputed weighted one-hot "scatter" matrix.

Numerics: every fp32 value on the matmul paths is represented as a bf16
hi/lo pair (hi = bf16(x), lo = bf16(x - hi)).  bf16 matmuls run at 1 PE
cycle/row vs fp32's 4, and the PE multiplies bf16 exactly with fp32
accumulation, so a 3-term product (hi*hi + hi*lo + lo*hi) is accurate to
~2^-18 relative -- far inside the 2e-2 harness gate.  Node features live in
DRAM as [node, 128] rows = (hi 64 | lo 64) bf16, so one 256B-row gather
feeds the edge matmul with both terms and the per-tile matmul computes the
hi- and lo- partial products in one pass (128-partition PSUM output).

Sharding: edges are sorted by destination and packed into tiles of <=128
edges covering <=32 whole destination nodes.  Real tiles are dealt
round-robin across the 8 cores so each core gets an equal share of edges.
Nodes are renumbered to (tile*32 + slot).  A core's edges land only in its
own node range, so no cross-core reduction is needed.  The evolving node
features are replicated via AllGather each iteration (chunked, so the
collective overlaps the tail of the GRU); iteration 0's features are
computed for ALL nodes on every core (lin0 is tiny), which removes one
AllGather entirely.
"""
import os
import sys

import numpy as np


def _ensure_path():
    for p in ("/opt/trn_rl_repo", os.path.expanduser("~/.axon_site/_ro/trn_rl_repo")):
        if os.path.isdir(p) and p not in sys.path:
            sys.path.insert(0, p)
    try:
        import concourse  # noqa: F401
    except ImportError as e:  # pragma: no cover
        raise ImportError(f"concourse (bass) not importable: {e}")


_ensure_path()

N_NODES, N_EDGES, IN_F, H = 10000, 50000, 32, 64
NCORES = 8
SLOTS = 32            # destination-node slots per tile
EPT = 128             # edge slots per tile
NCH = 5               # edge_attr channels (4) + constant channel for nn_b
T = 56                # tiles per core (fixed so the compiled NEFF is shape-stable)
NTILES = NCORES * T   # 448
NC_COLS = T * SLOTS   # padded nodes per core (1792)
NPAD = NCORES * NC_COLS
CHUNK = 512
GATHER_CHUNKS = 8
N_SWDGE_QUEUES = 4
QW = NCH * SLOTS      # 160 Q columns per tile
FB = 2 * H            # 128 bf16 feature bytes-row: hi|lo


def _chunks():
    out = []
    c0 = 0
    while c0 < NC_COLS:
        w = min(CHUNK, NC_COLS - c0)
        out.append((c0, w))
        c0 += w
    return out


# ----------------------------------------------------------------------------
# device program
# ----------------------------------------------------------------------------
_NC_CACHE = {}


def _get_nc():
    if "nc" in _NC_CACHE:
        return _NC_CACHE["nc"]
    import concourse.bacc as bacc
    import concourse.mybir as mybir
    import concourse.tile as tile

    dt = mybir.dt
    f32, i16, bf16 = dt.float32, dt.int16, dt.bfloat16
    AF = mybir.ActivationFunctionType
    ALU = mybir.AluOpType

    nc = bacc.Bacc(
        "TRN2",
        target_bir_lowering=False,
        debug=False,
        enable_asserts=False,
        num_devices=NCORES,
        num_swdge_queues=N_SWDGE_QUEUES,
    )

    qh_in = nc.dram_tensor("qh_in", [128, T * QW], bf16, kind="ExternalInput").ap()
    ql_in = nc.dram_tensor("ql_in", [128, T * QW], bf16, kind="ExternalInput").ap()
    idx_in = nc.dram_tensor("idx_in", [128, T * 8], i16, kind="ExternalInput").ap()
    xs_in = nc.dram_tensor("xs_in", [64, NPAD], bf16, kind="ExternalInput").ap()
    xso_in = nc.dram_tensor("xso_in", [64, NC_COLS], bf16, kind="ExternalInput").ap()
    l0_in = nc.dram_tensor("l0_in", [64, 128], bf16, kind="ExternalInput").ap()
    ws_in = nc.dram_tensor("ws_in", [128, NCH * 128], bf16, kind="ExternalInput").ap()
    root_in = nc.dram_tensor("root_in", [64, 128], bf16, kind="ExternalInput").ap()
    gru_in = nc.dram_tensor("gru_in", [64, 768], bf16, kind="ExternalInput").ap()
    bias_in = nc.dram_tensor("bias_in", [64, 8], f32, kind="ExternalInput").ap()
    ident_in = nc.dram_tensor("ident_in", [64, 64], bf16, kind="ExternalInput").ap()
    identf_in = nc.dram_tensor("identf_in", [64, 64], f32, kind="ExternalInput").ap()
    out_ext = nc.dram_tensor("out_sl", [NC_COLS, H], f32, kind="ExternalOutput").ap()

    chunks = _chunks()

    with tile.TileContext(nc) as tc:
        with tc.tile_pool(name="const", bufs=1) as const, \
             tc.tile_pool(name="work", bufs=1) as work, \
             tc.tile_pool(name="small", bufs=2) as small, \
             tc.tile_pool(name="ps", bufs=2, space="PSUM") as ps, \
             tc.tile_pool(name="aggp", bufs=2, space="PSUM") as aggp, \
             tc.tile_pool(name="grup", bufs=2, space="PSUM") as grup, \
             tc.tile_pool(name="tpp", bufs=2, space="PSUM") as tpp, \
             tc.tile_pool(name="dram", bufs=1, space="DRAM") as dram:

            qh_sb = const.tile([128, T * QW], bf16, name="qh_sb")
            ql_sb = const.tile([128, T * QW], bf16, name="ql_sb")
            idx_sb = const.tile([128, T * 8], i16, name="idx_sb")
            xs_sb = const.tile([64, NPAD], bf16, name="xs_sb")
            xso_sb = const.tile([64, NC_COLS], bf16, name="xso_sb")
            l0_sb = const.tile([64, 128], bf16, name="l0_sb")
            ws_sb = const.tile([128, NCH * 128], bf16, name="ws_sb")
            root_sb = const.tile([64, 128], bf16, name="root_sb")
            gru_sb = const.tile([64, 768], bf16, name="gru_sb")
            bias_sb = const.tile([64, 8], f32, name="bias_sb")
            ident_sb = const.tile([64, 64], bf16, name="ident_sb")
            identf_sb = const.tile([64, 64], f32, name="identf_sb")

            for sb_t, in_t in (
                (xso_sb, xso_in), (l0_sb, l0_in), (bias_sb, bias_in),
                (ident_sb, ident_in), (identf_sb, identf_in),
                (xs_sb, xs_in), (idx_sb, idx_in),
                (gru_sb, gru_in), (root_sb, root_in), (ws_sb, ws_in),
                (qh_sb, qh_in), (ql_sb, ql_in),
            ):
                nc.sync.dma_start(sb_t[:], in_t[:])

            l0b = bias_sb[:, 0:1]
            convb = bias_sb[:, 1:2]
            br = bias_sb[:, 2:3]
            bz = bias_sb[:, 3:4]
            bnih = bias_sb[:, 4:5]
            bnhh = bias_sb[:, 5:6]

            h32a = work.tile([64, NC_COLS], f32, name="h32a")
            h32b = work.tile([64, NC_COLS], f32, name="h32b")
            hha = work.tile([64, NC_COLS], bf16, name="hha")
            hla = work.tile([64, NC_COLS], bf16, name="hla")
            hhb = work.tile([64, NC_COLS], bf16, name="hhb")
            hlb = work.tile([64, NC_COLS], bf16, name="hlb")
            mh_sb = work.tile([64, NC_COLS], bf16, name="mh_sb")
            ml_sb = work.tile([64, NC_COLS], bf16, name="ml_sb")
            g_sb = work.tile([128, T * FB], bf16, name="g_sb")
            sh_sb = work.tile([128, T * QW], bf16, name="sh_sb")
            sl_sb = work.tile([128, T * QW], bf16, name="sl_sb")
            rowb = work.tile([128, NC_COLS], bf16, name="rowb")
            rowf = work.tile([128, (NC_COLS // 128) * H], f32, name="rowf")

            feat0 = dram.tile([NPAD, FB], bf16, name="feat0")
            agins = [dram.tile([NC_COLS, FB], bf16, name=f"agin{i}") for i in (1, 2)]
            agouts = [
                dram.tile([NPAD, FB], bf16, addr_space="Shared", name=f"agout{i}")
                for i in (1, 2)
            ]

            # ---- iteration 0: out0 = relu(x @ lin0_w + lin0_b) --------------
            # Full-table pass: every core computes out0 for ALL nodes and
            # writes the bf16 hi|lo feature table locally -- no collective.
            for c in range(NPAD // CHUNK):
                sl = slice(c * CHUNK, (c + 1) * CHUNK)
                p0 = grup.tile([64, CHUNK], f32, tag="g", name=f"l0f{c}")
                nc.tensor.matmul(p0[:], l0_sb[:, 0:64], xs_sb[:, sl],
                                 start=True, stop=False)
                nc.tensor.matmul(p0[:], l0_sb[:, 64:128], xs_sb[:, sl],
                                 start=False, stop=True)
                o32 = small.tile([64, CHUNK], f32, tag="o32", name=f"o32_{c}")
                nc.scalar.activation(o32[:], p0[:], AF.Relu, bias=l0b)
                oh = small.tile([64, CHUNK], bf16, tag="oh", name=f"oh_{c}")
                ol = small.tile([64, CHUNK], bf16, tag="ol", name=f"ol_{c}")
                nc.vector.tensor_copy(oh[:], o32[:])
                nc.vector.tensor_sub(ol[:], o32[:], oh[:])
                tp = tpp.tile([128, 512], bf16, tag="tp", name=f"t0_{c}")
                for b in range(CHUNK // 128):
                    bs = slice(b * 128, (b + 1) * 128)
                    nc.tensor.transpose(tp[:, b * 128:b * 128 + 64],
                                        oh[:, bs], ident_sb[:])
                    nc.tensor.transpose(tp[:, b * 128 + 64:(b + 1) * 128],
                                        ol[:, bs], ident_sb[:])
                rf = small.tile([128, 512], bf16, tag="rf", name=f"rf_{c}")
                if c % 2 == 0:
                    nc.scalar.copy(rf[:], tp[:])
                else:
                    nc.vector.tensor_copy(rf[:], tp[:])
                nc.sync.dma_start(
                    feat0[:].rearrange("(t p) f -> p t f", p=128)[:, 4 * c:4 * c + 4, :],
                    rf[:].rearrange("p (t f) -> p t f", f=FB),
                )
            # Own-slice pass (fp32 h and its bf16 split for root/GRU inputs).
            for k, (c0, w) in enumerate(chunks):
                sl = slice(c0, c0 + w)
                p1 = grup.tile([64, CHUNK], f32, tag="g", name=f"l0o{k}")
                nc.tensor.matmul(p1[:, :w], l0_sb[:, 0:64], xso_sb[:, sl],
                                 start=True, stop=False)
                nc.tensor.matmul(p1[:, :w], l0_sb[:, 64:128], xso_sb[:, sl],
                                 start=False, stop=True)
                nc.scalar.activation(h32a[:, sl], p1[:, :w], AF.Relu, bias=l0b)
                nc.vector.tensor_copy(hha[:, sl], h32a[:, sl])
                nc.vector.tensor_sub(hla[:, sl], h32a[:, sl], hha[:, sl])

            def edge_phase(it):
                src_dram = feat0 if it == 1 else agouts[it - 2]
                gsz = T // GATHER_CHUNKS
                for gc in range(GATHER_CHUNKS):
                    nc.gpsimd.dma_gather(
                        g_sb[:, gc * gsz * FB:(gc + 1) * gsz * FB].rearrange(
                            "p (t o) -> p t o", o=FB
                        ),
                        src_dram[:],
                        idx_sb[:, gc * gsz * 8:(gc + 1) * gsz * 8],
                        gsz * EPT,
                        gsz * EPT,
                        FB,
                        queue_num=gc % N_SWDGE_QUEUES,
                    )
                # mm1: S = G^T @ (Qh + Ql) per tile; PSUM partitions 0:64 get
                # the hi-feature products, 64:128 the lo-feature products.
                t = 0
                while t < T:
                    ntl = min(3, T - t)
                    s_ps = ps.tile([128, 3 * QW], f32, tag="s", name=f"sps{it}_{t}")
                    for j in range(ntl):
                        nc.tensor.matmul(
                            s_ps[:, j * QW:(j + 1) * QW],
                            g_sb[:, (t + j) * FB:(t + j + 1) * FB],
                            qh_sb[:, (t + j) * QW:(t + j + 1) * QW],
                            start=True, stop=False,
                        )
                        nc.tensor.matmul(
                            s_ps[:, j * QW:(j + 1) * QW],
                            g_sb[:, (t + j) * FB:(t + j + 1) * FB],
                            ql_sb[:, (t + j) * QW:(t + j + 1) * QW],
                            start=False, stop=True,
                        )
                    nc.scalar.copy(sh_sb[:, t * QW:(t + ntl) * QW],
                                   s_ps[:, :ntl * QW])
                    nc.vector.tensor_sub(sl_sb[:, t * QW:(t + ntl) * QW],
                                         s_ps[:, :ntl * QW],
                                         sh_sb[:, t * QW:(t + ntl) * QW])
                    t += ntl

            sh_re = sh_sb[:].rearrange("p (t c s) -> p t c s", c=NCH, s=SLOTS)
            sl_re = sl_sb[:].rearrange("p (t c s) -> p t c s", c=NCH, s=SLOTS)

            def dense_chunk(it, k, c0, w, h32, hh, hl, hn32, hhn, hln):
                sl = slice(c0, c0 + w)
                t0, tn = c0 // SLOTS, w // SLOTS
                # mm2 (+root) for this chunk's 16 tiles
                agg = aggp.tile([64, CHUNK], f32, tag="agg", name=f"agg{it}_{k}")
                for c in range(NCH):
                    nc.tensor.matmul(agg[:, :w], ws_sb[:, c * 128:c * 128 + 64],
                                     sh_re[:, t0:t0 + tn, c, :],
                                     start=(c == 0), stop=False)
                    nc.tensor.matmul(agg[:, :w], ws_sb[:, c * 128:c * 128 + 64],
                                     sl_re[:, t0:t0 + tn, c, :],
                                     start=False, stop=False)
                    nc.tensor.matmul(agg[:, :w], ws_sb[:, c * 128 + 64:c * 128 + 128],
                                     sh_re[:, t0:t0 + tn, c, :],
                                     start=False, stop=False)
                nc.tensor.matmul(agg[:, :w], root_sb[:, 0:64], hh[:, sl],
                                 start=False, stop=False)
                nc.tensor.matmul(agg[:, :w], root_sb[:, 0:64], hl[:, sl],
                                 start=False, stop=False)
                nc.tensor.matmul(agg[:, :w], root_sb[:, 64:128], hh[:, sl],
                                 start=False, stop=True)
                # m = relu(agg + conv_b), split hi/lo
                m32 = small.tile([64, CHUNK], f32, tag="m32", name=f"m32_{it}{k}")
                nc.scalar.activation(m32[:, :w], agg[:, :w], AF.Relu, bias=convb)
                nc.vector.tensor_copy(mh_sb[:, sl], m32[:, :w])
                nc.vector.tensor_sub(ml_sb[:, sl], m32[:, :w], mh_sb[:, sl])

                def gate(psname, ihc, hhc, last3=False):
                    p = grup.tile([64, CHUNK], f32, tag="g", name=psname)
                    nc.tensor.matmul(p[:, :w], gru_sb[:, ihc:ihc + 64],
                                     mh_sb[:, sl], start=True, stop=False)
                    nc.tensor.matmul(p[:, :w], gru_sb[:, ihc:ihc + 64],
                                     ml_sb[:, sl], start=False, stop=False)
                    nc.tensor.matmul(p[:, :w], gru_sb[:, ihc + 64:ihc + 128],
                                     mh_sb[:, sl], start=False,
                                     stop=(hhc is None))
                    if hhc is not None:
                        nc.tensor.matmul(p[:, :w], gru_sb[:, hhc:hhc + 64],
                                         hh[:, sl], start=False, stop=False)
                        nc.tensor.matmul(p[:, :w], gru_sb[:, hhc:hhc + 64],
                                         hl[:, sl], start=False, stop=False)
                        nc.tensor.matmul(p[:, :w], gru_sb[:, hhc + 64:hhc + 128],
                                         hh[:, sl], start=False, stop=True)
                    return p

                def hgate(psname, hhc):
                    p = grup.tile([64, CHUNK], f32, tag="g", name=psname)
                    nc.tensor.matmul(p[:, :w], gru_sb[:, hhc:hhc + 64],
                                     hh[:, sl], start=True, stop=False)
                    nc.tensor.matmul(p[:, :w], gru_sb[:, hhc:hhc + 64],
                                     hl[:, sl], start=False, stop=False)
                    nc.tensor.matmul(p[:, :w], gru_sb[:, hhc + 64:hhc + 128],
                                     hh[:, sl], start=False, stop=True)
                    return p

                r_ps = gate(f"r{it}_{k}", 0, 128)
                r_sb = small.tile([64, CHUNK], f32, tag="rsb", name=f"rs{it}{k}")
                nc.scalar.activation(r_sb[:, :w], r_ps[:, :w], AF.Sigmoid, bias=br)
                z_ps = gate(f"z{it}_{k}", 256, 384)
                z_sb = small.tile([64, CHUNK], f32, tag="zsb", name=f"zs{it}{k}")
                nc.scalar.activation(z_sb[:, :w], z_ps[:, :w], AF.Sigmoid, bias=bz)
                n1 = gate(f"n1{it}_{k}", 512, None)
                n2 = hgate(f"n2{it}_{k}", 640)
                # tmp = (n2 + b_hh_n) * r
                tmp = small.tile([64, CHUNK], f32, tag="tmp", name=f"tmp{it}{k}")
                nc.vector.scalar_tensor_tensor(
                    tmp[:, :w], n2[:, :w], bnhh, r_sb[:, :w], ALU.add, ALU.mult
                )
                pre = small.tile([64, CHUNK], f32, tag="pre", name=f"pre{it}{k}")
                nc.vector.tensor_add(pre[:, :w], n1[:, :w], tmp[:, :w])
                nsb = small.tile([64, CHUNK], f32, tag="nsb", name=f"nsb{it}{k}")
                nc.scalar.activation(nsb[:, :w], pre[:, :w], AF.Tanh, bias=bnih)
                # h' = n + z * (h - n)
                dd = small.tile([64, CHUNK], f32, tag="dd", name=f"dd{it}{k}")
                nc.vector.tensor_sub(dd[:, :w], h32[:, sl], nsb[:, :w])
                t4 = small.tile([64, CHUNK], f32, tag="t4", name=f"t4{it}{k}")
                nc.vector.tensor_mul(t4[:, :w], z_sb[:, :w], dd[:, :w])
                nc.vector.tensor_add(hn32[:, sl], nsb[:, :w], t4[:, :w])
                if it < 3:
                    # bf16 split of the new h for the next iteration
                    nc.scalar.copy(hhn[:, sl], hn32[:, sl])
                    nc.vector.tensor_sub(hln[:, sl], hn32[:, sl], hhn[:, sl])

            def writeback_chunk(it, k, c0, w, hhn, hln, hn32):
                nb = w // 128
                if it < 3:
                    tp = tpp.tile([128, 512], bf16, tag="tp", name=f"w{it}_{k}")
                    for b in range(nb):
                        bs = slice(c0 + b * 128, c0 + (b + 1) * 128)
                        nc.tensor.transpose(tp[:, b * 128:b * 128 + 64],
                                            hhn[:, bs], ident_sb[:])
                        nc.tensor.transpose(tp[:, b * 128 + 64:(b + 1) * 128],
                                            hln[:, bs], ident_sb[:])
                    if k % 2 == 0:
                        nc.scalar.copy(rowb[:, c0:c0 + w], tp[:, :nb * 128])
                    else:
                        nc.vector.tensor_copy(rowb[:, c0:c0 + w], tp[:, :nb * 128])
                    agin, agout = agins[it - 1], agouts[it - 1]
                    nc.sync.dma_start(
                        agin[:].rearrange("(t p) f -> p t f", p=128)[
                            :, c0 // 128:c0 // 128 + nb, :],
                        rowb[:, c0:c0 + w].rearrange("p (t f) -> p t f", f=FB),
                    )
                    if c0 + w == NC_COLS:
                        # Shared DRAM allows a single writer instruction, so
                        # the AllGather fires once, after the last chunk DMA.
                        nc.gpsimd.collective_compute(
                            "AllGather",
                            mybir.AluOpType.bypass,
                            replica_groups=[list(range(NCORES))],
                            ins=[agin[:]],
                            outs=[agout[:]],
                        )
                else:
                    # final iteration: write fp32 h to the external output
                    tpf = tpp.tile([128, 256], f32, tag="tp", name=f"wf_{k}")
                    for b in range(nb):
                        bs = slice(c0 + b * 128, c0 + (b + 1) * 128)
                        nc.tensor.transpose(
                            tpf[:, b * 64:(b + 1) * 64], hn32[:, bs], identf_sb[:])
                    if k % 2 == 0:
                        nc.scalar.copy(rowf[:, (c0 // 128) * H:(c0 // 128 + nb) * H],
                                       tpf[:, :nb * 64])
                    else:
                        nc.vector.tensor_copy(
                            rowf[:, (c0 // 128) * H:(c0 // 128 + nb) * H],
                            tpf[:, :nb * 64])

            h32, hn32 = h32a, h32b
            cur = (hha, hla, hhb, hlb)
            for it in (1, 2, 3):
                hh, hl, hhn, hln = cur
                edge_phase(it)
                for k, (c0, w) in enumerate(chunks):
                    dense_chunk(it, k, c0, w, h32, hh, hl, hn32, hhn, hln)
                    writeback_chunk(it, k, c0, w, hhn, hln, hn32)
                h32, hn32 = hn32, h32
                cur = (hhn, hln, hh, hl)

            nc.sync.dma_start(
                out_ext[:].rearrange("(t p) o -> p t o", p=128),
                rowf[:].rearrange("p (t o) -> p t o", o=H),
            )

    nc.compile()
    _NC_CACHE["nc"] = nc
    return nc


# ----------------------------------------------------------------------------
# host-side graph preprocessing (pure data layout, no model FLOPs)
# ----------------------------------------------------------------------------
def _bf16_rne(x):
    """Round fp32 -> bf16 (round-to-nearest-even). Returns (f32val, uint16bits)."""
    u = np.asarray(x, np.float32).view(np.uint32)
    r = (u + 0x7FFF + ((u >> 16) & 1)) & 0xFFFF0000
    return r.view(np.float32), (r >> 16).astype(np.uint16)


def _bf16_split(x):
    """x ~= hi + lo with both bf16. Returns (hi_f32, lo_f32, hi_u16, lo_u16)."""
    x = np.ascontiguousarray(np.asarray(x, np.float32))
    hi_f, hi_u = _bf16_rne(x)
    lo_f, lo_u = _bf16_rne(x - hi_f)
    return hi_f, lo_f, hi_u, lo_u


def _pack(edge_index, edge_attr):
    src = np.asarray(edge_index[0]).astype(np.int64)
    dst = np.asarray(edge_index[1]).astype(np.int64)
    ea = np.asarray(edge_attr, np.float32)
    order = np.argsort(dst, kind="stable")
    ssrc, sea = src[order], ea[order]
    deg = np.bincount(dst, minlength=N_NODES)
    starts = np.zeros(N_NODES + 1, np.int64)
    starts[1:] = np.cumsum(deg)
    uniq = np.flatnonzero(deg)
    zs = np.flatnonzero(deg == 0)
    node_seq = np.concatenate([uniq, zs])

    raw_tiles = [[]]
    ce = 0
    for nd in node_seq:
        d = int(deg[nd])
        assert d <= EPT, f"node degree {d} exceeds edge tile capacity"
        if len(raw_tiles[-1]) >= SLOTS or ce + d > EPT:
            raw_tiles.append([])
            ce = 0
        raw_tiles[-1].append(int(nd))
        ce += d
    assert len(raw_tiles) <= NTILES, f"need {len(raw_tiles)} tiles > {NTILES}"
    # Distribute real tiles round-robin across the 8 cores so every core gets
    # an equal share of real edges (a contiguous split leaves the last core
    # nearly all padding, which skews its runtime and stalls the collectives).
    tiles_nodes = [[] for _ in range(NTILES)]
    for i, nodes in enumerate(raw_tiles):
        core, j = i % NCORES, i // NCORES
        tiles_nodes[core * T + j] = nodes

    perm = np.empty(N_NODES, np.int64)
    for t, nodes in enumerate(tiles_nodes):
        for j, nd in enumerate(nodes):
            perm[nd] = t * SLOTS + j

    q = np.zeros((NTILES, EPT, NCH, SLOTS), np.float32)
    # Padding gather slots must NOT all point at row 0: thousands of reads of
    # one 256B row serialize on a single HBM bank.  Padding gets spread
    # distinct rows in the core's own slice.
    srcslot = np.full((NTILES, EPT), -1, np.int16)
    for t, nodes in enumerate(tiles_nodes):
        e = 0
        for j, nd in enumerate(nodes):
            s0, s1 = int(starts[nd]), int(starts[nd + 1])
            ne = s1 - s0
            if ne:
                q[t, e:e + ne, 0:4, j] = sea[s0:s1]
                q[t, e:e + ne, 4, j] = 1.0
                srcslot[t, e:e + ne] = perm[ssrc[s0:s1]].astype(np.int16)
                e += ne
    for k in range(NCORES):
        base = k * NC_COLS
        block = srcslot[k * T:(k + 1) * T].reshape(-1)
        holes = np.flatnonzero(block < 0)
        block[holes] = base + np.arange(holes.size) % NC_COLS
        srcslot[k * T:(k + 1) * T] = block.reshape(T, EPT)

    _, _, qh_u, ql_u = _bf16_split(q)
    qhs, qls, idxs = [], [], []
    i_arange = np.arange(T * EPT)
    for k in range(NCORES):
        def qlay(qu):
            qt = qu[k * T:(k + 1) * T]
            return np.ascontiguousarray(qt.transpose(1, 0, 2, 3)).reshape(
                128, T * QW)
        qhs.append(qlay(qh_u.reshape(NTILES, EPT, NCH, SLOTS)))
        qls.append(qlay(ql_u.reshape(NTILES, EPT, NCH, SLOTS)))
        flat = srcslot[k * T:(k + 1) * T].reshape(-1)
        ia = np.zeros((128, T * 8), np.int16)
        # the index list is read per 16-partition group by each of the 8
        # GPSIMD cores on HW -> replicate it into every group
        for g in range(8):
            ia[g * 16 + i_arange % 16, i_arange // 16] = flat
        idxs.append(ia)
    return qhs, qls, idxs, perm


def _prep_inputs(inputs):
    x = np.asarray(inputs["x"], np.float32)
    qhs, qls, idxs, perm = _pack(inputs["edge_index"], inputs["edge_attr"])

    x_pad = np.zeros((NPAD, IN_F), np.float32)
    x_pad[perm] = x
    xt = np.ascontiguousarray(x_pad.T)                      # [32, NPAD]
    _, _, xh_u, xl_u = _bf16_split(xt)
    xs_full = np.concatenate([xh_u, xl_u], axis=0)          # [64, NPAD] u16
    xsos = [
        np.ascontiguousarray(xs_full[:, k * NC_COLS:(k + 1) * NC_COLS])
        for k in range(NCORES)
    ]

    def stack2(a):                                          # [m,n] -> [2m,n]
        return np.concatenate([a, a], axis=0)

    lin0_w = np.asarray(inputs["lin0_w"], np.float32)       # [32, 64]
    _, _, w0h, w0l = _bf16_split(lin0_w)
    l0 = np.zeros((64, 128), np.uint16)
    l0[:, 0:64] = stack2(w0h)
    l0[:, 64:128] = stack2(w0l)

    nw = np.asarray(inputs["nn_w"], np.float32)
    ws = np.zeros((128, NCH * 128), np.uint16)
    for c in range(NCH):
        wc = (nw[c].reshape(H, H) if c < 4
              else np.asarray(inputs["nn_b"], np.float32).reshape(H, H))
        _, _, wch, wcl = _bf16_split(wc)
        ws[:, c * 128:c * 128 + 64] = stack2(wch)
        ws[:, c * 128 + 64:c * 128 + 128] = stack2(wcl)

    root_w = np.asarray(inputs["root_w"], np.float32)
    _, _, rh, rl = _bf16_split(root_w)
    root = np.zeros((64, 128), np.uint16)
    root[:, 0:64] = rh
    root[:, 64:128] = rl

    wih_t = np.ascontiguousarray(np.asarray(inputs["gru_w_ih"], np.float32).T)
    whh_t = np.ascontiguousarray(np.asarray(inputs["gru_w_hh"], np.float32).T)
    _, _, wih_h, wih_l = _bf16_split(wih_t)                 # [64, 192]
    _, _, whh_h, whh_l = _bf16_split(whh_t)
    gru = np.zeros((64, 768), np.uint16)
    for gi, g0 in enumerate((0, 64, 128)):                  # r, z, n
        base = gi * 256
        gs = slice(g0, g0 + 64)
        gru[:, base + 0:base + 64] = wih_h[:, gs]
        gru[:, base + 64:base + 128] = wih_l[:, gs]
        gru[:, base + 128:base + 192] = whh_h[:, gs]
        gru[:, base + 192:base + 256] = whh_l[:, gs]

    b_ih = np.asarray(inputs["gru_b_ih"], np.float32)
    b_hh = np.asarray(inputs["gru_b_hh"], np.float32)
    bias_pack = np.zeros((64, 8), np.float32)
    bias_pack[:, 0] = np.asarray(inputs["lin0_b"], np.float32)
    bias_pack[:, 1] = np.asarray(inputs["conv_b"], np.float32)
    bias_pack[:, 2] = (b_ih + b_hh)[0:64]
    bias_pack[:, 3] = (b_ih + b_hh)[64:128]
    bias_pack[:, 4] = b_ih[128:192]
    bias_pack[:, 5] = b_hh[128:192]
    identf = np.eye(64, dtype=np.float32)
    _, ident_u = _bf16_rne(identf)

    in_maps = []
    for k in range(NCORES):
        in_maps.append(
            {
                "qh_in": qhs[k],
                "ql_in": qls[k],
                "idx_in": idxs[k],
                "xs_in": xs_full,
                "xso_in": xsos[k],
                "l0_in": l0,
                "ws_in": ws,
                "root_in": root,
                "gru_in": gru,
                "bias_in": bias_pack,
                "ident_in": ident_u,
                "identf_in": identf,
            }
        )
    return in_maps, perm


def _assemble(results, perm):
    full = np.concatenate([results[k]["out_sl"] for k in range(NCORES)], axis=0)
    return np.ascontiguousarray(full[perm]).astype(np.float32)


def kernel(**inputs) -> np.ndarray:
    in_maps, perm = _prep_inputs(inputs)
    nc = _get_nc()
    if os.environ.get("BASS_KERNEL_SIM"):
        results = _run_sim(nc, in_maps)
    else:
        from concourse import bass_utils

        res = bass_utils.run_bass_kernel_spmd(
            nc, in_maps, core_ids=list(range(NCORES))
        )
        results = res.results
    return _assemble(results, perm)


def _run_sim(nc, in_maps):
    from concourse.bass_interp import MultiCoreSim

    sim = MultiCoreSim(nc, num_cores=NCORES, trace=False)
    for k, core in sim.cores.items():
        for name, arr in in_maps[k].items():
            core.tensor(name)[:] = arr
    sim.simulate(check_with_hw=False)
    out = []
    for k in range(NCORES):
        out.append({"out_sl": np.array(sim.cores[k].tensor("out_sl"))})
    return out


if __name__ == "__main__":
    rng = np.random.default_rng(0)
    demo = {
        "x": rng.standard_normal((N_NODES, IN_F), dtype=np.float32),
        "edge_index": rng.integers(0, N_NODES, (2, N_EDGES)).astype(np.int32),
        "edge_attr": rng.random((N_EDGES, 4), dtype=np.float32),
        "lin0_w": rng.standard_normal((IN_F, H), dtype=np.float32) * 0.1,
        "lin0_b": np.zeros(H, np.float32),
        "nn_w": rng.standard_normal((4, H * H), dtype=np.float32) * 0.05,
        "nn_b": np.zeros(H * H, np.float32),
        "root_w": rng.standard_normal((H, H), dtype=np.float32) * 0.1,
        "conv_b": np.zeros(H, np.float32),
        "gru_w_ih": rng.standard_normal((3 * H, H), dtype=np.float32) * 0.1,
        "gru_w_hh": rng.standard_normal((3 * H, H), dtype=np.float32) * 0.1,
        "gru_b_ih": np.zeros(3 * H, np.float32),
        "gru_b_hh": np.zeros(3 * H, np.float32),
    }
    out = kernel(**demo)
    print("kernel output", out.shape, out.dtype, float(np.abs(out).mean()))


# revision 21
# speedup vs baseline: 1.7020x; 1.0453x over previous
"""Bass/Trainium2 kernel for nn_Net_19602230739296 (NNConv + GRU message passing GNN).

Algorithm (mathematically equivalent to the reference):
  theta[e] = (edge_attr[e] @ nn_w + nn_b).reshape(H, H) is never materialized.
  msg[e]   = sum_c ea'[e,c] * (out[src_e] @ W_c)   with ea' = [edge_attr, 1],
             W_c = nn_w[c].reshape(H,H) for c<4, W_4 = nn_b.reshape(H,H).
  agg^T    = sum_c W_c^T @ (G^T @ Q_c)  per 128-edge tile, where G = out[src]
             (gathered rows) and Q_c[e, slot] = ea'[e,c] * [dst_e == slot-node]
             is a host-precomputed weighted one-hot "scatter" matrix.

Numerics: every fp32 value on the matmul paths is represented as a bf16
hi/lo pair (hi = bf16(x), lo = bf16(x - hi)).  bf16 matmuls run at 1 PE
cycle/row vs fp32's 4, and the PE multiplies bf16 exactly with fp32
accumulation, so a 3-term product (hi*hi + hi*lo + lo*hi) is accurate to
~2^-18 relative -- far inside the 2e-2 harness gate.  Node features live in
DRAM as [node, 128] rows = (hi 64 | lo 64) bf16, so one 256B-row gather
feeds the edge matmul with both terms and the per-tile matmul computes the
hi- and lo- partial products in one pass (128-partition PSUM output).

Sharding: edges are sorted by destination and packed into tiles of <=128
edges covering <=32 whole destination nodes.  Real tiles are dealt
round-robin across the 8 cores so each core gets an equal share of edges.
Nodes are renumbered to (tile*32 + slot).  A core's edges land only in its
own node range, so no cross-core reduction is needed.  The evolving node
features are replicated via AllGather each iteration (chunked, so the
collective overlaps the tail of the GRU); iteration 0's features are
computed for ALL nodes on every core (lin0 is tiny), which removes one
AllGather entirely.
"""
import os
import sys

import numpy as np


def _ensure_path():
    for p in ("/opt/trn_rl_repo", os.path.expanduser("~/.axon_site/_ro/trn_rl_repo")):
        if os.path.isdir(p) and p not in sys.path:
            sys.path.insert(0, p)
    try:
        import concourse  # noqa: F401
    except ImportError as e:  # pragma: no cover
        raise ImportError(f"concourse (bass) not importable: {e}")


_ensure_path()

N_NODES, N_EDGES, IN_F, H = 10000, 50000, 32, 64
NCORES = 8
SLOTS = 32            # destination-node slots per tile
EPT = 128             # edge slots per tile
NCH = 5               # edge_attr channels (4) + constant channel for nn_b
T = 56                # tiles per core (fixed so the compiled NEFF is shape-stable)
NTILES = NCORES * T   # 448
NC_COLS = T * SLOTS   # padded nodes per core (1792)
NPAD = NCORES * NC_COLS
CHUNK = 512
GATHER_CHUNKS = 8
N_SWDGE_QUEUES = 4
QW = NCH * SLOTS      # 160 Q columns per tile
FB = 2 * H            # 128 bf16 feature bytes-row: hi|lo


def _chunks():
    out = []
    c0 = 0
    while c0 < NC_COLS:
        w = min(CHUNK, NC_COLS - c0)
        out.append((c0, w))
        c0 += w
    return out


# ----------------------------------------------------------------------------
# device program
# ----------------------------------------------------------------------------
_NC_CACHE = {}


def _get_nc():
    if "nc" in _NC_CACHE:
        return _NC_CACHE["nc"]
    import concourse.bacc as bacc
    import concourse.mybir as mybir
    import concourse.tile as tile

    dt = mybir.dt
    f32, i16, bf16 = dt.float32, dt.int16, dt.bfloat16
    AF = mybir.ActivationFunctionType
    ALU = mybir.AluOpType

    nc = bacc.Bacc(
        "TRN2",
        target_bir_lowering=False,
        debug=False,
        enable_asserts=False,
        num_devices=NCORES,
        num_swdge_queues=N_SWDGE_QUEUES,
    )

    qh_in = nc.dram_tensor("qh_in", [128, T * QW], bf16, kind="ExternalInput").ap()
    ql_in = nc.dram_tensor("ql_in", [128, T * QW], bf16, kind="ExternalInput").ap()
    idx_in = nc.dram_tensor("idx_in", [128, T * 8], i16, kind="ExternalInput").ap()
    xs_in = nc.dram_tensor("xs_in", [64, NPAD], bf16, kind="ExternalInput").ap()
    xso_in = nc.dram_tensor("xso_in", [64, NC_COLS], bf16, kind="ExternalInput").ap()
    l0_in = nc.dram_tensor("l0_in", [64, 128], bf16, kind="ExternalInput").ap()
    ws_in = nc.dram_tensor("ws_in", [128, NCH * 128], bf16, kind="ExternalInput").ap()
    root_in = nc.dram_tensor("root_in", [128, 64], bf16, kind="ExternalInput").ap()
    gru_in = nc.dram_tensor("gru_in", [128, 384], bf16, kind="ExternalInput").ap()
    bias_in = nc.dram_tensor("bias_in", [128, 8], f32, kind="ExternalInput").ap()
    ident_in = nc.dram_tensor("ident_in", [128, 128], bf16, kind="ExternalInput").ap()
    identf_in = nc.dram_tensor("identf_in", [64, 64], f32, kind="ExternalInput").ap()
    out_ext = nc.dram_tensor("out_sl", [NC_COLS, H], f32, kind="ExternalOutput").ap()

    chunks = _chunks()

    with tile.TileContext(nc) as tc:
        with tc.tile_pool(name="const", bufs=1) as const, \
             tc.tile_pool(name="work", bufs=1) as work, \
             tc.tile_pool(name="small", bufs=2) as small, \
             tc.tile_pool(name="ps", bufs=2, space="PSUM") as ps, \
             tc.tile_pool(name="aggp", bufs=2, space="PSUM") as aggp, \
             tc.tile_pool(name="grz", bufs=1, space="PSUM") as grz, \
             tc.tile_pool(name="gn", bufs=2, space="PSUM") as gn, \
             tc.tile_pool(name="tpp", bufs=1, space="PSUM") as tpp, \
             tc.tile_pool(name="dram", bufs=1, space="DRAM") as dram:

            qh_sb = const.tile([128, T * QW], bf16, name="qh_sb")
            ql_sb = const.tile([128, T * QW], bf16, name="ql_sb")
            idx_sb = const.tile([128, T * 8], i16, name="idx_sb")
            xs_sb = const.tile([64, NPAD], bf16, name="xs_sb")
            xso_sb = const.tile([64, NC_COLS], bf16, name="xso_sb")
            l0_sb = const.tile([64, 128], bf16, name="l0_sb")
            ws_sb = const.tile([128, NCH * 128], bf16, name="ws_sb")
            root_sb = const.tile([128, 64], bf16, name="root_sb")
            gru_sb = const.tile([128, 384], bf16, name="gru_sb")
            bias_sb = const.tile([128, 8], f32, name="bias_sb")
            ident_sb = const.tile([128, 128], bf16, name="ident_sb")
            identf_sb = const.tile([64, 64], f32, name="identf_sb")

            for sb_t, in_t in (
                (xso_sb, xso_in), (l0_sb, l0_in), (bias_sb, bias_in),
                (ident_sb, ident_in), (identf_sb, identf_in),
                (xs_sb, xs_in), (idx_sb, idx_in),
                (gru_sb, gru_in), (root_sb, root_in), (ws_sb, ws_in),
                (qh_sb, qh_in), (ql_sb, ql_in),
            ):
                nc.sync.dma_start(sb_t[:], in_t[:])

            l0b = bias_sb[0:64, 0:1]
            convb = bias_sb[0:64, 1:2]
            brz = bias_sb[:, 2:3]          # [br ; bz] stacked on 128 partitions
            bnih = bias_sb[0:64, 4:5]
            bnhh = bias_sb[0:64, 5:6]

            h32a = work.tile([128, NC_COLS], f32, name="h32a")
            h32b = work.tile([128, NC_COLS], f32, name="h32b")
            # h bf16 split, stacked [hi;lo] and swapped [lo;hi] on 128 parts
            hs1a = work.tile([128, NC_COLS], bf16, name="hs1a")
            hs2a = work.tile([128, NC_COLS], bf16, name="hs2a")
            hs1b = work.tile([128, NC_COLS], bf16, name="hs1b")
            hs2b = work.tile([128, NC_COLS], bf16, name="hs2b")
            g_sb = work.tile([128, T * FB], bf16, name="g_sb")
            sh_sb = work.tile([128, T * QW], bf16, name="sh_sb")
            sl_sb = work.tile([128, T * QW], bf16, name="sl_sb")
            rowb = work.tile([128, NC_COLS], bf16, name="rowb")
            rowf = work.tile([128, (NC_COLS // 128) * H], f32, name="rowf")
            warm_sb = work.tile([128, 64], bf16, name="warm_sb")

            feat0 = dram.tile([NPAD, FB], bf16, name="feat0")
            agins = [dram.tile([NC_COLS, FB], bf16, name=f"agin{i}") for i in (1, 2)]
            agouts = [
                dram.tile([NPAD, FB], bf16, addr_space="Shared", name=f"agout{i}")
                for i in (1, 2)
            ]

            # ---- iteration 0: out0 = relu(x @ lin0_w + lin0_b) --------------
            # Full-table pass: every core computes out0 for ALL nodes and
            # writes the bf16 hi|lo feature table locally -- no collective.
            for c in range(NPAD // CHUNK):
                sl = slice(c * CHUNK, (c + 1) * CHUNK)
                p0 = gn.tile([64, CHUNK], f32, tag="gn", name=f"l0f{c}")
                nc.tensor.matmul(p0[:], l0_sb[:, 0:64], xs_sb[:, sl],
                                 start=True, stop=False)
                nc.tensor.matmul(p0[:], l0_sb[:, 64:128], xs_sb[:, sl],
                                 start=False, stop=True)
                o32 = small.tile([128, CHUNK], f32, tag="o32", name=f"o32_{c}")
                nc.scalar.activation(o32[0:64, :], p0[:], AF.Relu, bias=l0b)
                osk = small.tile([128, CHUNK], bf16, tag="osk", name=f"osk_{c}")
                nc.vector.tensor_copy(osk[0:64, :], o32[0:64, :])
                nc.vector.tensor_sub(osk[64:128, :], o32[0:64, :], osk[0:64, :])
                tp = tpp.tile([128, 512], bf16, tag="tp", name=f"t0_{c}")
                for b in range(CHUNK // 128):
                    bs = slice(b * 128, (b + 1) * 128)
                    nc.tensor.transpose(tp[:, b * 128:(b + 1) * 128],
                                        osk[:, bs], ident_sb[:])
                rf = small.tile([128, 512], bf16, tag="rf", name=f"rf_{c}")
                if c % 2 == 0:
                    nc.scalar.copy(rf[:], tp[:])
                else:
                    nc.vector.tensor_copy(rf[:], tp[:])
                nc.sync.dma_start(
                    feat0[:].rearrange("(t p) f -> p t f", p=128)[:, 4 * c:4 * c + 4, :],
                    rf[:].rearrange("p (t f) -> p t f", f=FB),
                )
            # Own-slice pass (fp32 h and its bf16 split for root/GRU inputs).
            for k, (c0, w) in enumerate(chunks):
                sl = slice(c0, c0 + w)
                p1 = gn.tile([64, CHUNK], f32, tag="gn", name=f"l0o{k}")
                nc.tensor.matmul(p1[:, :w], l0_sb[:, 0:64], xso_sb[:, sl],
                                 start=True, stop=False)
                nc.tensor.matmul(p1[:, :w], l0_sb[:, 64:128], xso_sb[:, sl],
                                 start=False, stop=True)
                nc.scalar.activation(h32a[0:64, sl], p1[:, :w], AF.Relu, bias=l0b)
                nc.vector.tensor_copy(hs1a[0:64, sl], h32a[0:64, sl])
                nc.vector.tensor_sub(hs1a[64:128, sl], h32a[0:64, sl],
                                     hs1a[0:64, sl])
                nc.scalar.copy(hs2a[0:64, sl], hs1a[64:128, sl])
                nc.scalar.copy(hs2a[64:128, sl], hs1a[0:64, sl])

            def warm_chain(links, it):
                # Keep the PE's HAM activity monitor from dropping to the
                # cold 1.2GHz clock while the engine waits on the collective:
                # a dependency chain of tiny matmuls paces ~1 PE op per µs.
                for i in range(links):
                    wp = gn.tile([64, CHUNK], f32, tag="gn", name=f"wm{it}_{i}")
                    nc.tensor.matmul(wp[:, 0:64], ident_sb[:, 0:64],
                                     warm_sb[:], start=True, stop=True)
                    nc.vector.tensor_copy(warm_sb[0:64, :], wp[:, 0:64])
                    nc.vector.tensor_copy(warm_sb[64:128, :], wp[:, 0:64])

            def edge_phase(it):
                src_dram = feat0 if it == 1 else agouts[it - 2]
                gsz = T // GATHER_CHUNKS
                for gc in range(GATHER_CHUNKS):
                    nc.gpsimd.dma_gather(
                        g_sb[:, gc * gsz * FB:(gc + 1) * gsz * FB].rearrange(
                            "p (t o) -> p t o", o=FB
                        ),
                        src_dram[:],
                        idx_sb[:, gc * gsz * 8:(gc + 1) * gsz * 8],
                        gsz * EPT,
                        gsz * EPT,
                        FB,
                        queue_num=gc % N_SWDGE_QUEUES,
                    )
                # mm1: S = G^T @ (Qh + Ql) per tile; PSUM partitions 0:64 get
                # the hi-feature products, 64:128 the lo-feature products.
                t = 0
                while t < T:
                    ntl = min(3, T - t)
                    s_ps = ps.tile([128, 3 * QW], f32, tag="s", name=f"sps{it}_{t}")
                    for j in range(ntl):
                        nc.tensor.matmul(
                            s_ps[:, j * QW:(j + 1) * QW],
                            g_sb[:, (t + j) * FB:(t + j + 1) * FB],
                            qh_sb[:, (t + j) * QW:(t + j + 1) * QW],
                            start=True, stop=False,
                        )
                        nc.tensor.matmul(
                            s_ps[:, j * QW:(j + 1) * QW],
                            g_sb[:, (t + j) * FB:(t + j + 1) * FB],
                            ql_sb[:, (t + j) * QW:(t + j + 1) * QW],
                            start=False, stop=True,
                        )
                    nc.scalar.copy(sh_sb[:, t * QW:(t + ntl) * QW],
                                   s_ps[:, :ntl * QW])
                    nc.vector.tensor_sub(sl_sb[:, t * QW:(t + ntl) * QW],
                                         s_ps[:, :ntl * QW],
                                         sh_sb[:, t * QW:(t + ntl) * QW])
                    t += ntl

            sh_re = sh_sb[:].rearrange("p (t c s) -> p t c s", c=NCH, s=SLOTS)
            sl_re = sl_sb[:].rearrange("p (t c s) -> p t c s", c=NCH, s=SLOTS)

            def dense_chunk(it, k, c0, w, h32, hs1, hs2, hn32, hs1n, hs2n):
                sl = slice(c0, c0 + w)
                t0, tn = c0 // SLOTS, w // SLOTS
                # mm2 (+root) for this chunk's 16 tiles
                agg = aggp.tile([64, CHUNK], f32, tag="agg", name=f"agg{it}_{k}")
                for c in range(NCH):
                    nc.tensor.matmul(agg[:, :w], ws_sb[:, c * 128:c * 128 + 64],
                                     sh_re[:, t0:t0 + tn, c, :],
                                     start=(c == 0), stop=False)
                    nc.tensor.matmul(agg[:, :w], ws_sb[:, c * 128:c * 128 + 64],
                                     sl_re[:, t0:t0 + tn, c, :],
                                     start=False, stop=False)
                    nc.tensor.matmul(agg[:, :w], ws_sb[:, c * 128 + 64:c * 128 + 128],
                                     sh_re[:, t0:t0 + tn, c, :],
                                     start=False, stop=False)
                nc.tensor.matmul(agg[:, :w], root_sb[:], hs1[:, sl],
                                 start=False, stop=False)
                nc.tensor.matmul(agg[:, :w], root_sb[:], hs2[:, sl],
                                 start=False, stop=True)
                # m = relu(agg + conv_b); bf16 stacks [mh;ml] and [ml;mh]
                m32 = small.tile([128, CHUNK], f32, tag="m32", name=f"m32_{it}{k}")
                nc.scalar.activation(m32[0:64, :w], agg[:, :w], AF.Relu, bias=convb)
                ms1 = small.tile([128, CHUNK], bf16, tag="ms1", name=f"ms1_{it}{k}")
                ms2 = small.tile([128, CHUNK], bf16, tag="ms2", name=f"ms2_{it}{k}")
                nc.vector.tensor_copy(ms1[0:64, :w], m32[0:64, :w])
                nc.vector.tensor_sub(ms1[64:128, :w], m32[0:64, :w], ms1[0:64, :w])
                nc.scalar.copy(ms2[0:64, :w], ms1[64:128, :w])
                nc.scalar.copy(ms2[64:128, :w], ms1[0:64, :w])

                # r|z stacked on 128 partitions: 4 FWL matmuls
                rz = grz.tile([128, CHUNK], f32, tag="rz", name=f"rz{it}_{k}")
                nc.tensor.matmul(rz[:, :w], gru_sb[:, 0:128], ms1[:, :w],
                                 start=True, stop=False)
                nc.tensor.matmul(rz[:, :w], gru_sb[:, 0:128], ms2[:, :w],
                                 start=False, stop=False)
                nc.tensor.matmul(rz[:, :w], gru_sb[:, 128:256], hs1[:, sl],
                                 start=False, stop=False)
                nc.tensor.matmul(rz[:, :w], gru_sb[:, 128:256], hs2[:, sl],
                                 start=False, stop=True)
                rz_sb = small.tile([128, CHUNK], f32, tag="rzsb", name=f"rzs{it}{k}")
                nc.scalar.activation(rz_sb[:, :w], rz[:, :w], AF.Sigmoid, bias=brz)
                n1 = gn.tile([64, CHUNK], f32, tag="gn", name=f"n1{it}_{k}")
                nc.tensor.matmul(n1[:, :w], gru_sb[:, 256:320], ms1[:, :w],
                                 start=True, stop=False)
                nc.tensor.matmul(n1[:, :w], gru_sb[:, 256:320], ms2[:, :w],
                                 start=False, stop=True)
                n2 = gn.tile([64, CHUNK], f32, tag="gn", name=f"n2{it}_{k}")
                nc.tensor.matmul(n2[:, :w], gru_sb[:, 320:384], hs1[:, sl],
                                 start=True, stop=False)
                nc.tensor.matmul(n2[:, :w], gru_sb[:, 320:384], hs2[:, sl],
                                 start=False, stop=True)
                # tmp = (n2 + b_hh_n) * r
                tmp = small.tile([64, CHUNK], f32, tag="tmp", name=f"tmp{it}{k}")
                nc.vector.scalar_tensor_tensor(
                    tmp[:, :w], n2[:, :w], bnhh, rz_sb[0:64, :w], ALU.add, ALU.mult
                )
                pre = small.tile([64, CHUNK], f32, tag="pre", name=f"pre{it}{k}")
                nc.vector.tensor_add(pre[:, :w], n1[:, :w], tmp[:, :w])
                nsb = small.tile([128, CHUNK], f32, tag="nsb", name=f"nsb{it}{k}")
                nc.scalar.activation(nsb[0:64, :w], pre[:, :w], AF.Tanh, bias=bnih)
                # h' = n + z * (h - n)
                dd = small.tile([128, CHUNK], f32, tag="dd", name=f"dd{it}{k}")
                nc.vector.tensor_sub(dd[0:64, :w], h32[0:64, sl], nsb[0:64, :w])
                t4 = small.tile([128, CHUNK], f32, tag="t4", name=f"t4{it}{k}")
                z_sb = small.tile([128, CHUNK], f32, tag="zsb", name=f"zsb{it}{k}")
                nc.scalar.copy(z_sb[0:64, :w], rz_sb[64:128, :w])
                nc.vector.tensor_mul(t4[0:64, :w], z_sb[0:64, :w], dd[0:64, :w])
                nc.vector.tensor_add(hn32[0:64, sl], nsb[0:64, :w], t4[0:64, :w])
                if it < 3:
                    # bf16 split (and swap) of the new h for the next iteration
                    nc.scalar.copy(hs1n[0:64, sl], hn32[0:64, sl])
                    nc.vector.tensor_sub(hs1n[64:128, sl], hn32[0:64, sl],
                                         hs1n[0:64, sl])
                    nc.scalar.copy(hs2n[0:64, sl], hs1n[64:128, sl])
                    nc.vector.tensor_copy(hs2n[64:128, sl], hs1n[0:64, sl])

            def writeback_chunk(it, k, c0, w, hs1n, hn32):
                nb = w // 128
                if it < 3:
                    tp = tpp.tile([128, 512], bf16, tag="tp", name=f"w{it}_{k}")
                    for b in range(nb):
                        bs = slice(c0 + b * 128, c0 + (b + 1) * 128)
                        nc.tensor.transpose(tp[:, b * 128:(b + 1) * 128],
                                            hs1n[:, bs], ident_sb[:])
                    if k % 2 == 0:
                        nc.scalar.copy(rowb[:, c0:c0 + w], tp[:, :nb * 128])
                    else:
                        nc.vector.tensor_copy(rowb[:, c0:c0 + w], tp[:, :nb * 128])
                    agin, agout = agins[it - 1], agouts[it - 1]
                    nc.sync.dma_start(
                        agin[:].rearrange("(t p) f -> p t f", p=128)[
                            :, c0 // 128:c0 // 128 + nb, :],
                        rowb[:, c0:c0 + w].rearrange("p (t f) -> p t f", f=FB),
                    )
                    if c0 + w == NC_COLS:
                        # Shared DRAM allows a single writer instruction, so
                        # the AllGather fires once, after the last chunk DMA.
                        nc.gpsimd.collective_compute(
                            "AllGather",
                            mybir.AluOpType.bypass,
                            replica_groups=[list(range(NCORES))],
                            ins=[agin[:]],
                            outs=[agout[:]],
                        )
                else:
                    # final iteration: write fp32 h to the external output
                    tpf = tpp.tile([128, 256], f32, tag="tp", name=f"wf_{k}")
                    for b in range(nb):
                        bs = slice(c0 + b * 128, c0 + (b + 1) * 128)
                        nc.tensor.transpose(
                            tpf[:, b * 64:(b + 1) * 64], hn32[0:64, bs],
                            identf_sb[:])
                    if k % 2 == 0:
                        nc.scalar.copy(rowf[:, (c0 // 128) * H:(c0 // 128 + nb) * H],
                                       tpf[:, :nb * 64])
                    else:
                        nc.vector.tensor_copy(
                            rowf[:, (c0 // 128) * H:(c0 // 128 + nb) * H],
                            tpf[:, :nb * 64])

            nc.vector.tensor_copy(warm_sb[:], ident_sb[:, 0:64])
            h32, hn32 = h32a, h32b
            cur = (hs1a, hs2a, hs1b, hs2b)
            for it in (1, 2, 3):
                hs1, hs2, hs1n, hs2n = cur
                warm_chain(4 if it == 1 else 24, it)
                edge_phase(it)
                for k, (c0, w) in enumerate(chunks):
                    dense_chunk(it, k, c0, w, h32, hs1, hs2, hn32, hs1n, hs2n)
                    writeback_chunk(it, k, c0, w, hs1n, hn32)
                h32, hn32 = hn32, h32
                cur = (hs1n, hs2n, hs1, hs2)

            nc.sync.dma_start(
                out_ext[:].rearrange("(t p) o -> p t o", p=128),
                rowf[:].rearrange("p (t o) -> p t o", o=H),
            )

    nc.compile()
    _NC_CACHE["nc"] = nc
    return nc


# ----------------------------------------------------------------------------
# host-side graph preprocessing (pure data layout, no model FLOPs)
# ----------------------------------------------------------------------------
def _bf16_rne(x):
    """Round fp32 -> bf16 (round-to-nearest-even). Returns (f32val, uint16bits)."""
    u = np.asarray(x, np.float32).view(np.uint32)
    r = (u + 0x7FFF + ((u >> 16) & 1)) & 0xFFFF0000
    return r.view(np.float32), (r >> 16).astype(np.uint16)


def _bf16_split(x):
    """x ~= hi + lo with both bf16. Returns (hi_f32, lo_f32, hi_u16, lo_u16)."""
    x = np.ascontiguousarray(np.asarray(x, np.float32))
    hi_f, hi_u = _bf16_rne(x)
    lo_f, lo_u = _bf16_rne(x - hi_f)
    return hi_f, lo_f, hi_u, lo_u


def _pack(edge_index, edge_attr):
    src = np.asarray(edge_index[0]).astype(np.int64)
    dst = np.asarray(edge_index[1]).astype(np.int64)
    ea = np.asarray(edge_attr, np.float32)
    order = np.argsort(dst, kind="stable")
    ssrc, sea = src[order], ea[order]
    deg = np.bincount(dst, minlength=N_NODES)
    starts = np.zeros(N_NODES + 1, np.int64)
    starts[1:] = np.cumsum(deg)
    uniq = np.flatnonzero(deg)
    zs = np.flatnonzero(deg == 0)
    node_seq = np.concatenate([uniq, zs])

    raw_tiles = [[]]
    ce = 0
    for nd in node_seq:
        d = int(deg[nd])
        assert d <= EPT, f"node degree {d} exceeds edge tile capacity"
        if len(raw_tiles[-1]) >= SLOTS or ce + d > EPT:
            raw_tiles.append([])
            ce = 0
        raw_tiles[-1].append(int(nd))
        ce += d
    assert len(raw_tiles) <= NTILES, f"need {len(raw_tiles)} tiles > {NTILES}"
    # Distribute real tiles round-robin across the 8 cores so every core gets
    # an equal share of real edges (a contiguous split leaves the last core
    # nearly all padding, which skews its runtime and stalls the collectives).
    tiles_nodes = [[] for _ in range(NTILES)]
    for i, nodes in enumerate(raw_tiles):
        core, j = i % NCORES, i // NCORES
        tiles_nodes[core * T + j] = nodes

    perm = np.empty(N_NODES, np.int64)
    for t, nodes in enumerate(tiles_nodes):
        for j, nd in enumerate(nodes):
            perm[nd] = t * SLOTS + j

    q = np.zeros((NTILES, EPT, NCH, SLOTS), np.float32)
    # Padding gather slots must NOT all point at row 0: thousands of reads of
    # one 256B row serialize on a single HBM bank.  Padding gets spread
    # distinct rows in the core's own slice.
    srcslot = np.full((NTILES, EPT), -1, np.int16)
    for t, nodes in enumerate(tiles_nodes):
        e = 0
        for j, nd in enumerate(nodes):
            s0, s1 = int(starts[nd]), int(starts[nd + 1])
            ne = s1 - s0
            if ne:
                q[t, e:e + ne, 0:4, j] = sea[s0:s1]
                q[t, e:e + ne, 4, j] = 1.0
                srcslot[t, e:e + ne] = perm[ssrc[s0:s1]].astype(np.int16)
                e += ne
    for k in range(NCORES):
        base = k * NC_COLS
        block = srcslot[k * T:(k + 1) * T].reshape(-1)
        holes = np.flatnonzero(block < 0)
        block[holes] = base + np.arange(holes.size) % NC_COLS
        srcslot[k * T:(k + 1) * T] = block.reshape(T, EPT)

    _, _, qh_u, ql_u = _bf16_split(q)
    qhs, qls, idxs = [], [], []
    i_arange = np.arange(T * EPT)
    for k in range(NCORES):
        def qlay(qu):
            qt = qu[k * T:(k + 1) * T]
            return np.ascontiguousarray(qt.transpose(1, 0, 2, 3)).reshape(
                128, T * QW)
        qhs.append(qlay(qh_u.reshape(NTILES, EPT, NCH, SLOTS)))
        qls.append(qlay(ql_u.reshape(NTILES, EPT, NCH, SLOTS)))
        flat = srcslot[k * T:(k + 1) * T].reshape(-1)
        ia = np.zeros((128, T * 8), np.int16)
        # the index list is read per 16-partition group by each of the 8
        # GPSIMD cores on HW -> replicate it into every group
        for g in range(8):
            ia[g * 16 + i_arange % 16, i_arange // 16] = flat
        idxs.append(ia)
    return qhs, qls, idxs, perm


def _prep_inputs(inputs):
    x = np.asarray(inputs["x"], np.float32)
    qhs, qls, idxs, perm = _pack(inputs["edge_index"], inputs["edge_attr"])

    x_pad = np.zeros((NPAD, IN_F), np.float32)
    x_pad[perm] = x
    xt = np.ascontiguousarray(x_pad.T)                      # [32, NPAD]
    _, _, xh_u, xl_u = _bf16_split(xt)
    xs_full = np.concatenate([xh_u, xl_u], axis=0)          # [64, NPAD] u16
    xsos = [
        np.ascontiguousarray(xs_full[:, k * NC_COLS:(k + 1) * NC_COLS])
        for k in range(NCORES)
    ]

    def stack2(a):                                          # [m,n] -> [2m,n]
        return np.concatenate([a, a], axis=0)

    lin0_w = np.asarray(inputs["lin0_w"], np.float32)       # [32, 64]
    _, _, w0h, w0l = _bf16_split(lin0_w)
    l0 = np.zeros((64, 128), np.uint16)
    l0[:, 0:64] = stack2(w0h)
    l0[:, 64:128] = stack2(w0l)

    nw = np.asarray(inputs["nn_w"], np.float32)
    ws = np.zeros((128, NCH * 128), np.uint16)
    for c in range(NCH):
        wc = (nw[c].reshape(H, H) if c < 4
              else np.asarray(inputs["nn_b"], np.float32).reshape(H, H))
        _, _, wch, wcl = _bf16_split(wc)
        ws[:, c * 128:c * 128 + 64] = stack2(wch)
        ws[:, c * 128 + 64:c * 128 + 128] = stack2(wcl)

    root_w = np.asarray(inputs["root_w"], np.float32)
    _, _, rh, rl = _bf16_split(root_w)
    root = np.concatenate([rh, rl], axis=0)                 # [128, 64]

    wih_t = np.ascontiguousarray(np.asarray(inputs["gru_w_ih"], np.float32).T)
    whh_t = np.ascontiguousarray(np.asarray(inputs["gru_w_hh"], np.float32).T)
    _, _, wih_h, wih_l = _bf16_split(wih_t)                 # [64, 192]
    _, _, whh_h, whh_l = _bf16_split(whh_t)
    gru = np.zeros((128, 384), np.uint16)
    gru[0:64, 0:128] = wih_h[:, 0:128]                      # Vih_rz
    gru[64:128, 0:128] = wih_l[:, 0:128]
    gru[0:64, 128:256] = whh_h[:, 0:128]                    # Vhh_rz
    gru[64:128, 128:256] = whh_l[:, 0:128]
    gru[0:64, 256:320] = wih_h[:, 128:192]                  # Vih_n
    gru[64:128, 256:320] = wih_l[:, 128:192]
    gru[0:64, 320:384] = whh_h[:, 128:192]                  # Vhh_n
    gru[64:128, 320:384] = whh_l[:, 128:192]

    b_ih = np.asarray(inputs["gru_b_ih"], np.float32)
    b_hh = np.asarray(inputs["gru_b_hh"], np.float32)
    bias_pack = np.zeros((128, 8), np.float32)
    bias_pack[0:64, 0] = np.asarray(inputs["lin0_b"], np.float32)
    bias_pack[0:64, 1] = np.asarray(inputs["conv_b"], np.float32)
    bias_pack[0:64, 2] = (b_ih + b_hh)[0:64]                # b_r
    bias_pack[64:128, 2] = (b_ih + b_hh)[64:128]            # b_z
    bias_pack[0:64, 4] = b_ih[128:192]
    bias_pack[0:64, 5] = b_hh[128:192]
    identf = np.eye(64, dtype=np.float32)
    _, ident_u = _bf16_rne(np.eye(128, dtype=np.float32))

    in_maps = []
    for k in range(NCORES):
        in_maps.append(
            {
                "qh_in": qhs[k],
                "ql_in": qls[k],
                "idx_in": idxs[k],
                "xs_in": xs_full,
                "xso_in": xsos[k],
                "l0_in": l0,
                "ws_in": ws,
                "root_in": root,
                "gru_in": gru,
                "bias_in": bias_pack,
                "ident_in": ident_u,
                "identf_in": identf,
            }
        )
    return in_maps, perm


def _assemble(results, perm):
    full = np.concatenate([results[k]["out_sl"] for k in range(NCORES)], axis=0)
    return np.ascontiguousarray(full[perm]).astype(np.float32)


def kernel(**inputs) -> np.ndarray:
    in_maps, perm = _prep_inputs(inputs)
    nc = _get_nc()
    if os.environ.get("BASS_KERNEL_SIM"):
        results = _run_sim(nc, in_maps)
    else:
        from concourse import bass_utils

        res = bass_utils.run_bass_kernel_spmd(
            nc, in_maps, core_ids=list(range(NCORES))
        )
        results = res.results
    return _assemble(results, perm)


def _run_sim(nc, in_maps):
    from concourse.bass_interp import MultiCoreSim

    sim = MultiCoreSim(nc, num_cores=NCORES, trace=False)
    for k, core in sim.cores.items():
        for name, arr in in_maps[k].items():
            core.tensor(name)[:] = arr
    sim.simulate(check_with_hw=False)
    out = []
    for k in range(NCORES):
        out.append({"out_sl": np.array(sim.cores[k].tensor("out_sl"))})
    return out


if __name__ == "__main__":
    rng = np.random.default_rng(0)
    demo = {
        "x": rng.standard_normal((N_NODES, IN_F), dtype=np.float32),
        "edge_index": rng.integers(0, N_NODES, (2, N_EDGES)).astype(np.int32),
        "edge_attr": rng.random((N_EDGES, 4), dtype=np.float32),
        "lin0_w": rng.standard_normal((IN_F, H), dtype=np.float32) * 0.1,
        "lin0_b": np.zeros(H, np.float32),
        "nn_w": rng.standard_normal((4, H * H), dtype=np.float32) * 0.05,
        "nn_b": np.zeros(H * H, np.float32),
        "root_w": rng.standard_normal((H, H), dtype=np.float32) * 0.1,
        "conv_b": np.zeros(H, np.float32),
        "gru_w_ih": rng.standard_normal((3 * H, H), dtype=np.float32) * 0.1,
        "gru_w_hh": rng.standard_normal((3 * H, H), dtype=np.float32) * 0.1,
        "gru_b_ih": np.zeros(3 * H, np.float32),
        "gru_b_hh": np.zeros(3 * H, np.float32),
    }
    out = kernel(**demo)
    print("kernel output", out.shape, out.dtype, float(np.abs(out).mean()))


# revision 22
# speedup vs baseline: 1.7508x; 1.0287x over previous
"""Bass/Trainium2 kernel for nn_Net_19602230739296 (NNConv + GRU message passing GNN).

Algorithm (mathematically equivalent to the reference):
  theta[e] = (edge_attr[e] @ nn_w + nn_b).reshape(H, H) is never materialized.
  msg[e]   = sum_c ea'[e,c] * (out[src_e] @ W_c)   with ea' = [edge_attr, 1],
             W_c = nn_w[c].reshape(H,H) for c<4, W_4 = nn_b.reshape(H,H).
  agg^T    = sum_c W_c^T @ (G^T @ Q_c)  per 128-edge tile, where G = out[src]
             (gathered rows) and Q_c[e, slot] = ea'[e,c] * [dst_e == slot-node]
             is a host-precomputed weighted one-hot "scatter" matrix.

Numerics: every fp32 value on the matmul paths is represented as a bf16
hi/lo pair (hi = bf16(x), lo = bf16(x - hi)).  bf16 matmuls run at 1 PE
cycle/row vs fp32's 4, and the PE multiplies bf16 exactly with fp32
accumulation, so a 3-term product (hi*hi + hi*lo + lo*hi) is accurate to
~2^-18 relative -- far inside the 2e-2 harness gate.  Node features live in
DRAM as [node, 128] rows = (hi 64 | lo 64) bf16, so one 256B-row gather
feeds the edge matmul with both terms and the per-tile matmul computes the
hi- and lo- partial products in one pass (128-partition PSUM output).

Sharding: edges are sorted by destination and packed into tiles of <=128
edges covering <=32 whole destination nodes.  Real tiles are dealt
round-robin across the 8 cores so each core gets an equal share of edges.
Nodes are renumbered to (tile*32 + slot).  A core's edges land only in its
own node range, so no cross-core reduction is needed.  The evolving node
features are replicated via AllGather each iteration (chunked, so the
collective overlaps the tail of the GRU); iteration 0's features are
computed for ALL nodes on every core (lin0 is tiny), which removes one
AllGather entirely.
"""
import os
import sys

import numpy as np


def _ensure_path():
    for p in ("/opt/trn_rl_repo", os.path.expanduser("~/.axon_site/_ro/trn_rl_repo")):
        if os.path.isdir(p) and p not in sys.path:
            sys.path.insert(0, p)
    try:
        import concourse  # noqa: F401
    except ImportError as e:  # pragma: no cover
        raise ImportError(f"concourse (bass) not importable: {e}")


_ensure_path()

N_NODES, N_EDGES, IN_F, H = 10000, 50000, 32, 64
NCORES = 8
SLOTS = 24            # destination-node slots per tile
EPT = 128             # edge slots per tile
NCH = 5               # edge_attr channels (4) + constant channel for nn_b
T = 64                # tiles per core (fixed so the compiled NEFF is shape-stable)
NTILES = NCORES * T   # 448
NC_COLS = T * SLOTS   # padded nodes per core (1792)
NPAD = NCORES * NC_COLS
CHUNK = 384
GATHER_CHUNKS = 16
N_SWDGE_QUEUES = 4
QW = NCH * SLOTS      # 160 Q columns per tile
FB = 2 * H            # 128 bf16 feature bytes-row: hi|lo


def _chunks():
    out = []
    c0 = 0
    while c0 < NC_COLS:
        w = min(CHUNK, NC_COLS - c0)
        out.append((c0, w))
        c0 += w
    return out


# ----------------------------------------------------------------------------
# device program
# ----------------------------------------------------------------------------
_NC_CACHE = {}


def _get_nc():
    if "nc" in _NC_CACHE:
        return _NC_CACHE["nc"]
    import concourse.bacc as bacc
    import concourse.mybir as mybir
    import concourse.tile as tile

    dt = mybir.dt
    f32, i16, bf16 = dt.float32, dt.int16, dt.bfloat16
    AF = mybir.ActivationFunctionType
    ALU = mybir.AluOpType

    nc = bacc.Bacc(
        "TRN2",
        target_bir_lowering=False,
        debug=False,
        enable_asserts=False,
        num_devices=NCORES,
        num_swdge_queues=N_SWDGE_QUEUES,
    )

    qh_in = nc.dram_tensor("qh_in", [128, T * QW], bf16, kind="ExternalInput").ap()
    ql_in = nc.dram_tensor("ql_in", [128, T * QW], bf16, kind="ExternalInput").ap()
    idx_in = nc.dram_tensor("idx_in", [128, T * 8], i16, kind="ExternalInput").ap()
    xs_in = nc.dram_tensor("xs_in", [64, NPAD], bf16, kind="ExternalInput").ap()
    xso_in = nc.dram_tensor("xso_in", [64, NC_COLS], bf16, kind="ExternalInput").ap()
    l0_in = nc.dram_tensor("l0_in", [64, 128], bf16, kind="ExternalInput").ap()
    ws_in = nc.dram_tensor("ws_in", [128, NCH * 128], bf16, kind="ExternalInput").ap()
    root_in = nc.dram_tensor("root_in", [128, 64], bf16, kind="ExternalInput").ap()
    gru_in = nc.dram_tensor("gru_in", [128, 384], bf16, kind="ExternalInput").ap()
    bias_in = nc.dram_tensor("bias_in", [128, 8], f32, kind="ExternalInput").ap()
    ident_in = nc.dram_tensor("ident_in", [128, 128], bf16, kind="ExternalInput").ap()
    identf_in = nc.dram_tensor("identf_in", [64, 64], f32, kind="ExternalInput").ap()
    out_ext = nc.dram_tensor("out_sl", [NC_COLS, H], f32, kind="ExternalOutput").ap()

    chunks = _chunks()

    with tile.TileContext(nc) as tc:
        with tc.tile_pool(name="const", bufs=1) as const, \
             tc.tile_pool(name="work", bufs=1) as work, \
             tc.tile_pool(name="small", bufs=2) as small, \
             tc.tile_pool(name="ps", bufs=2, space="PSUM") as ps, \
             tc.tile_pool(name="aggp", bufs=2, space="PSUM") as aggp, \
             tc.tile_pool(name="grz", bufs=1, space="PSUM") as grz, \
             tc.tile_pool(name="gn", bufs=2, space="PSUM") as gn, \
             tc.tile_pool(name="tpp", bufs=1, space="PSUM") as tpp, \
             tc.tile_pool(name="dram", bufs=1, space="DRAM") as dram:

            qh_sb = const.tile([128, T * QW], bf16, name="qh_sb")
            ql_sb = const.tile([128, T * QW], bf16, name="ql_sb")
            idx_sb = const.tile([128, T * 8], i16, name="idx_sb")
            xs_sb = const.tile([64, NPAD], bf16, name="xs_sb")
            xso_sb = const.tile([64, NC_COLS], bf16, name="xso_sb")
            l0_sb = const.tile([64, 128], bf16, name="l0_sb")
            ws_sb = const.tile([128, NCH * 128], bf16, name="ws_sb")
            root_sb = const.tile([128, 64], bf16, name="root_sb")
            gru_sb = const.tile([128, 384], bf16, name="gru_sb")
            bias_sb = const.tile([128, 8], f32, name="bias_sb")
            ident_sb = const.tile([128, 128], bf16, name="ident_sb")
            identf_sb = const.tile([64, 64], f32, name="identf_sb")

            for sb_t, in_t in (
                (xso_sb, xso_in), (l0_sb, l0_in), (bias_sb, bias_in),
                (ident_sb, ident_in), (identf_sb, identf_in),
                (xs_sb, xs_in), (idx_sb, idx_in),
                (gru_sb, gru_in), (root_sb, root_in), (ws_sb, ws_in),
                (qh_sb, qh_in), (ql_sb, ql_in),
            ):
                nc.sync.dma_start(sb_t[:], in_t[:])

            l0b = bias_sb[0:64, 0:1]
            convb = bias_sb[0:64, 1:2]
            brz = bias_sb[:, 2:3]          # [br ; bz] stacked on 128 partitions
            bnih = bias_sb[0:64, 4:5]
            bnhh = bias_sb[0:64, 5:6]

            h32a = work.tile([128, NC_COLS], f32, name="h32a")
            h32b = work.tile([128, NC_COLS], f32, name="h32b")
            # h bf16 split, stacked [hi;lo] and swapped [lo;hi] on 128 parts
            hs1a = work.tile([128, NC_COLS], bf16, name="hs1a")
            hs2a = work.tile([128, NC_COLS], bf16, name="hs2a")
            hs1b = work.tile([128, NC_COLS], bf16, name="hs1b")
            hs2b = work.tile([128, NC_COLS], bf16, name="hs2b")
            g_sb = work.tile([128, T * FB], bf16, name="g_sb")
            sh_sb = work.tile([128, T * QW], bf16, name="sh_sb")
            sl_sb = work.tile([128, T * QW], bf16, name="sl_sb")
            rowb = work.tile([128, NC_COLS], bf16, name="rowb")
            rowf = work.tile([128, (NC_COLS // 128) * H], f32, name="rowf")
            warm_sb = work.tile([128, 64], bf16, name="warm_sb")

            feat0 = dram.tile([NPAD, FB], bf16, name="feat0")
            agins = [dram.tile([NC_COLS, FB], bf16, name=f"agin{i}") for i in (1, 2)]
            agouts = [
                dram.tile([NPAD, FB], bf16, addr_space="Shared", name=f"agout{i}")
                for i in (1, 2)
            ]

            # ---- iteration 0: out0 = relu(x @ lin0_w + lin0_b) --------------
            # Full-table pass: every core computes out0 for ALL nodes and
            # writes the bf16 hi|lo feature table locally -- no collective.
            for c in range(NPAD // CHUNK):
                sl = slice(c * CHUNK, (c + 1) * CHUNK)
                p0 = gn.tile([64, CHUNK], f32, tag="gn", name=f"l0f{c}")
                nc.tensor.matmul(p0[:], l0_sb[:, 0:64], xs_sb[:, sl],
                                 start=True, stop=False)
                nc.tensor.matmul(p0[:], l0_sb[:, 64:128], xs_sb[:, sl],
                                 start=False, stop=True)
                o32 = small.tile([128, CHUNK], f32, tag="o32", name=f"o32_{c}")
                nc.scalar.activation(o32[0:64, :], p0[:], AF.Relu, bias=l0b)
                osk = small.tile([128, CHUNK], bf16, tag="osk", name=f"osk_{c}")
                nc.vector.tensor_copy(osk[0:64, :], o32[0:64, :])
                nc.vector.tensor_sub(osk[64:128, :], o32[0:64, :], osk[0:64, :])
                tp = tpp.tile([128, 512], bf16, tag="tp", name=f"t0_{c}")
                for b in range(CHUNK // 128):
                    bs = slice(b * 128, (b + 1) * 128)
                    nc.tensor.transpose(tp[:, b * 128:(b + 1) * 128],
                                        osk[:, bs], ident_sb[:])
                nbc = CHUNK // 128
                rf = small.tile([128, CHUNK], bf16, tag="rf", name=f"rf_{c}")
                if c % 2 == 0:
                    nc.scalar.copy(rf[:], tp[:, :CHUNK])
                else:
                    nc.vector.tensor_copy(rf[:], tp[:, :CHUNK])
                nc.sync.dma_start(
                    feat0[:].rearrange("(t p) f -> p t f", p=128)[
                        :, nbc * c:nbc * c + nbc, :],
                    rf[:].rearrange("p (t f) -> p t f", f=FB),
                )
            # Own-slice pass (fp32 h and its bf16 split for root/GRU inputs).
            for k, (c0, w) in enumerate(chunks):
                sl = slice(c0, c0 + w)
                p1 = gn.tile([64, CHUNK], f32, tag="gn", name=f"l0o{k}")
                nc.tensor.matmul(p1[:, :w], l0_sb[:, 0:64], xso_sb[:, sl],
                                 start=True, stop=False)
                nc.tensor.matmul(p1[:, :w], l0_sb[:, 64:128], xso_sb[:, sl],
                                 start=False, stop=True)
                nc.scalar.activation(h32a[0:64, sl], p1[:, :w], AF.Relu, bias=l0b)
                nc.vector.tensor_copy(hs1a[0:64, sl], h32a[0:64, sl])
                nc.vector.tensor_sub(hs1a[64:128, sl], h32a[0:64, sl],
                                     hs1a[0:64, sl])
                nc.scalar.copy(hs2a[0:64, sl], hs1a[64:128, sl])
                nc.scalar.copy(hs2a[64:128, sl], hs1a[0:64, sl])

            def warm_chain(links, it):
                # Keep the PE's HAM activity monitor from dropping to the
                # cold 1.2GHz clock while the engine waits on the collective:
                # a dependency chain of tiny matmuls paces ~1 PE op per µs.
                for i in range(links):
                    wp = gn.tile([64, CHUNK], f32, tag="gn", name=f"wm{it}_{i}")
                    nc.tensor.matmul(wp[:, 0:64], ident_sb[:, 0:64],
                                     warm_sb[:], start=True, stop=True)
                    nc.vector.tensor_copy(warm_sb[0:64, :], wp[:, 0:64])
                    nc.vector.tensor_copy(warm_sb[64:128, :], wp[:, 0:64])

            def edge_phase(it):
                src_dram = feat0 if it == 1 else agouts[it - 2]
                gsz = T // GATHER_CHUNKS
                for gc in range(GATHER_CHUNKS):
                    nc.gpsimd.dma_gather(
                        g_sb[:, gc * gsz * FB:(gc + 1) * gsz * FB].rearrange(
                            "p (t o) -> p t o", o=FB
                        ),
                        src_dram[:],
                        idx_sb[:, gc * gsz * 8:(gc + 1) * gsz * 8],
                        gsz * EPT,
                        gsz * EPT,
                        FB,
                        queue_num=gc % N_SWDGE_QUEUES,
                    )
                # mm1: S = G^T @ (Qh + Ql) per tile; PSUM partitions 0:64 get
                # the hi-feature products, 64:128 the lo-feature products.
                t = 0
                while t < T:
                    ntl = min(3, T - t)
                    s_ps = ps.tile([128, 3 * QW], f32, tag="s", name=f"sps{it}_{t}")
                    for j in range(ntl):
                        nc.tensor.matmul(
                            s_ps[:, j * QW:(j + 1) * QW],
                            g_sb[:, (t + j) * FB:(t + j + 1) * FB],
                            qh_sb[:, (t + j) * QW:(t + j + 1) * QW],
                            start=True, stop=False,
                        )
                        nc.tensor.matmul(
                            s_ps[:, j * QW:(j + 1) * QW],
                            g_sb[:, (t + j) * FB:(t + j + 1) * FB],
                            ql_sb[:, (t + j) * QW:(t + j + 1) * QW],
                            start=False, stop=True,
                        )
                    nc.scalar.copy(sh_sb[:, t * QW:(t + ntl) * QW],
                                   s_ps[:, :ntl * QW])
                    nc.vector.tensor_sub(sl_sb[:, t * QW:(t + ntl) * QW],
                                         s_ps[:, :ntl * QW],
                                         sh_sb[:, t * QW:(t + ntl) * QW])
                    t += ntl

            sh_re = sh_sb[:].rearrange("p (t c s) -> p t c s", c=NCH, s=SLOTS)
            sl_re = sl_sb[:].rearrange("p (t c s) -> p t c s", c=NCH, s=SLOTS)

            def dense_chunk(it, k, c0, w, h32, hs1, hs2, hn32, hs1n, hs2n):
                sl = slice(c0, c0 + w)
                t0, tn = c0 // SLOTS, w // SLOTS
                # mm2 (+root) for this chunk's 16 tiles
                agg = aggp.tile([64, CHUNK], f32, tag="agg", name=f"agg{it}_{k}")
                for c in range(NCH):
                    nc.tensor.matmul(agg[:, :w], ws_sb[:, c * 128:c * 128 + 64],
                                     sh_re[:, t0:t0 + tn, c, :],
                                     start=(c == 0), stop=False)
                    nc.tensor.matmul(agg[:, :w], ws_sb[:, c * 128:c * 128 + 64],
                                     sl_re[:, t0:t0 + tn, c, :],
                                     start=False, stop=False)
                    nc.tensor.matmul(agg[:, :w], ws_sb[:, c * 128 + 64:c * 128 + 128],
                                     sh_re[:, t0:t0 + tn, c, :],
                                     start=False, stop=False)
                nc.tensor.matmul(agg[:, :w], root_sb[:], hs1[:, sl],
                                 start=False, stop=False)
                nc.tensor.matmul(agg[:, :w], root_sb[:], hs2[:, sl],
                                 start=False, stop=True)
                # m = relu(agg + conv_b); bf16 stacks [mh;ml] and [ml;mh]
                m32 = small.tile([128, CHUNK], f32, tag="m32", name=f"m32_{it}{k}")
                nc.scalar.activation(m32[0:64, :w], agg[:, :w], AF.Relu, bias=convb)
                ms1 = small.tile([128, CHUNK], bf16, tag="ms1", name=f"ms1_{it}{k}")
                ms2 = small.tile([128, CHUNK], bf16, tag="ms2", name=f"ms2_{it}{k}")
                nc.vector.tensor_copy(ms1[0:64, :w], m32[0:64, :w])
                nc.vector.tensor_sub(ms1[64:128, :w], m32[0:64, :w], ms1[0:64, :w])
                nc.scalar.copy(ms2[0:64, :w], ms1[64:128, :w])
                nc.scalar.copy(ms2[64:128, :w], ms1[0:64, :w])

                # r|z stacked on 128 partitions: 4 FWL matmuls
                rz = grz.tile([128, CHUNK], f32, tag="rz", name=f"rz{it}_{k}")
                nc.tensor.matmul(rz[:, :w], gru_sb[:, 0:128], ms1[:, :w],
                                 start=True, stop=False)
                nc.tensor.matmul(rz[:, :w], gru_sb[:, 0:128], ms2[:, :w],
                                 start=False, stop=False)
                nc.tensor.matmul(rz[:, :w], gru_sb[:, 128:256], hs1[:, sl],
                                 start=False, stop=False)
                nc.tensor.matmul(rz[:, :w], gru_sb[:, 128:256], hs2[:, sl],
                                 start=False, stop=True)
                rz_sb = small.tile([128, CHUNK], f32, tag="rzsb", name=f"rzs{it}{k}")
                nc.scalar.activation(rz_sb[:, :w], rz[:, :w], AF.Sigmoid, bias=brz)
                n1 = gn.tile([64, CHUNK], f32, tag="gn", name=f"n1{it}_{k}")
                nc.tensor.matmul(n1[:, :w], gru_sb[:, 256:320], ms1[:, :w],
                                 start=True, stop=False)
                nc.tensor.matmul(n1[:, :w], gru_sb[:, 256:320], ms2[:, :w],
                                 start=False, stop=True)
                n2 = gn.tile([64, CHUNK], f32, tag="gn", name=f"n2{it}_{k}")
                nc.tensor.matmul(n2[:, :w], gru_sb[:, 320:384], hs1[:, sl],
                                 start=True, stop=False)
                nc.tensor.matmul(n2[:, :w], gru_sb[:, 320:384], hs2[:, sl],
                                 start=False, stop=True)
                # tmp = (n2 + b_hh_n) * r
                tmp = small.tile([64, CHUNK], f32, tag="tmp", name=f"tmp{it}{k}")
                nc.vector.scalar_tensor_tensor(
                    tmp[:, :w], n2[:, :w], bnhh, rz_sb[0:64, :w], ALU.add, ALU.mult
                )
                pre = small.tile([64, CHUNK], f32, tag="pre", name=f"pre{it}{k}")
                nc.vector.tensor_add(pre[:, :w], n1[:, :w], tmp[:, :w])
                nsb = small.tile([128, CHUNK], f32, tag="nsb", name=f"nsb{it}{k}")
                nc.scalar.activation(nsb[0:64, :w], pre[:, :w], AF.Tanh, bias=bnih)
                # h' = n + z * (h - n)
                dd = small.tile([128, CHUNK], f32, tag="dd", name=f"dd{it}{k}")
                nc.vector.tensor_sub(dd[0:64, :w], h32[0:64, sl], nsb[0:64, :w])
                t4 = small.tile([128, CHUNK], f32, tag="t4", name=f"t4{it}{k}")
                z_sb = small.tile([128, CHUNK], f32, tag="zsb", name=f"zsb{it}{k}")
                nc.scalar.copy(z_sb[0:64, :w], rz_sb[64:128, :w])
                nc.vector.tensor_mul(t4[0:64, :w], z_sb[0:64, :w], dd[0:64, :w])
                nc.vector.tensor_add(hn32[0:64, sl], nsb[0:64, :w], t4[0:64, :w])
                if it < 3:
                    # bf16 split (and swap) of the new h for the next iteration
                    nc.scalar.copy(hs1n[0:64, sl], hn32[0:64, sl])
                    nc.vector.tensor_sub(hs1n[64:128, sl], hn32[0:64, sl],
                                         hs1n[0:64, sl])
                    nc.sync.dma_start(hs2n[0:64, sl], hs1n[64:128, sl])
                    nc.sync.dma_start(hs2n[64:128, sl], hs1n[0:64, sl])

            def writeback_chunk(it, k, c0, w, hs1n, hn32):
                nb = w // 128
                if it < 3:
                    tp = tpp.tile([128, 512], bf16, tag="tp", name=f"w{it}_{k}")
                    for b in range(nb):
                        bs = slice(c0 + b * 128, c0 + (b + 1) * 128)
                        nc.tensor.transpose(tp[:, b * 128:(b + 1) * 128],
                                            hs1n[:, bs], ident_sb[:])
                    if k % 2 == 0:
                        nc.scalar.copy(rowb[:, c0:c0 + w], tp[:, :nb * 128])
                    else:
                        nc.vector.tensor_copy(rowb[:, c0:c0 + w], tp[:, :nb * 128])
                    agin, agout = agins[it - 1], agouts[it - 1]
                    nc.sync.dma_start(
                        agin[:].rearrange("(t p) f -> p t f", p=128)[
                            :, c0 // 128:c0 // 128 + nb, :],
                        rowb[:, c0:c0 + w].rearrange("p (t f) -> p t f", f=FB),
                    )
                    if c0 + w == NC_COLS:
                        # Shared DRAM allows a single writer instruction, so
                        # the AllGather fires once, after the last chunk DMA.
                        nc.gpsimd.collective_compute(
                            "AllGather",
                            mybir.AluOpType.bypass,
                            replica_groups=[list(range(NCORES))],
                            ins=[agin[:]],
                            outs=[agout[:]],
                        )
                else:
                    # final iteration: write fp32 h to the external output
                    tpf = tpp.tile([128, 256], f32, tag="tp", name=f"wf_{k}")
                    for b in range(nb):
                        bs = slice(c0 + b * 128, c0 + (b + 1) * 128)
                        nc.tensor.transpose(
                            tpf[:, b * 64:(b + 1) * 64], hn32[0:64, bs],
                            identf_sb[:])
                    if k % 2 == 0:
                        nc.scalar.copy(rowf[:, (c0 // 128) * H:(c0 // 128 + nb) * H],
                                       tpf[:, :nb * 64])
                    else:
                        nc.vector.tensor_copy(
                            rowf[:, (c0 // 128) * H:(c0 // 128 + nb) * H],
                            tpf[:, :nb * 64])

            nc.vector.tensor_copy(warm_sb[:], ident_sb[:, 0:64])
            h32, hn32 = h32a, h32b
            cur = (hs1a, hs2a, hs1b, hs2b)
            for it in (1, 2, 3):
                hs1, hs2, hs1n, hs2n = cur
                warm_chain(8 if it == 1 else 56, it)
                edge_phase(it)
                for k, (c0, w) in enumerate(chunks):
                    dense_chunk(it, k, c0, w, h32, hs1, hs2, hn32, hs1n, hs2n)
                    writeback_chunk(it, k, c0, w, hs1n, hn32)
                h32, hn32 = hn32, h32
                cur = (hs1n, hs2n, hs1, hs2)

            nc.sync.dma_start(
                out_ext[:].rearrange("(t p) o -> p t o", p=128),
                rowf[:].rearrange("p (t o) -> p t o", o=H),
            )

    nc.compile()
    _NC_CACHE["nc"] = nc
    return nc


# ----------------------------------------------------------------------------
# host-side graph preprocessing (pure data layout, no model FLOPs)
# ----------------------------------------------------------------------------
def _bf16_rne(x):
    """Round fp32 -> bf16 (round-to-nearest-even). Returns (f32val, uint16bits)."""
    u = np.asarray(x, np.float32).view(np.uint32)
    r = (u + 0x7FFF + ((u >> 16) & 1)) & 0xFFFF0000
    return r.view(np.float32), (r >> 16).astype(np.uint16)


def _bf16_split(x):
    """x ~= hi + lo with both bf16. Returns (hi_f32, lo_f32, hi_u16, lo_u16)."""
    x = np.ascontiguousarray(np.asarray(x, np.float32))
    hi_f, hi_u = _bf16_rne(x)
    lo_f, lo_u = _bf16_rne(x - hi_f)
    return hi_f, lo_f, hi_u, lo_u


def _pack(edge_index, edge_attr):
    src = np.asarray(edge_index[0]).astype(np.int64)
    dst = np.asarray(edge_index[1]).astype(np.int64)
    ea = np.asarray(edge_attr, np.float32)
    order = np.argsort(dst, kind="stable")
    ssrc, sea = src[order], ea[order]
    deg = np.bincount(dst, minlength=N_NODES)
    starts = np.zeros(N_NODES + 1, np.int64)
    starts[1:] = np.cumsum(deg)
    uniq = np.flatnonzero(deg)
    zs = np.flatnonzero(deg == 0)
    node_seq = np.concatenate([uniq, zs])

    raw_tiles = [[]]
    ce = 0
    for nd in node_seq:
        d = int(deg[nd])
        assert d <= EPT, f"node degree {d} exceeds edge tile capacity"
        if len(raw_tiles[-1]) >= SLOTS or ce + d > EPT:
            raw_tiles.append([])
            ce = 0
        raw_tiles[-1].append(int(nd))
        ce += d
    assert len(raw_tiles) <= NTILES, f"need {len(raw_tiles)} tiles > {NTILES}"
    # Distribute real tiles round-robin across the 8 cores so every core gets
    # an equal share of real edges (a contiguous split leaves the last core
    # nearly all padding, which skews its runtime and stalls the collectives).
    tiles_nodes = [[] for _ in range(NTILES)]
    for i, nodes in enumerate(raw_tiles):
        core, j = i % NCORES, i // NCORES
        tiles_nodes[core * T + j] = nodes

    perm = np.empty(N_NODES, np.int64)
    for t, nodes in enumerate(tiles_nodes):
        for j, nd in enumerate(nodes):
            perm[nd] = t * SLOTS + j

    q = np.zeros((NTILES, EPT, NCH, SLOTS), np.float32)
    # Padding gather slots must NOT all point at row 0: thousands of reads of
    # one 256B row serialize on a single HBM bank.  Padding gets spread
    # distinct rows in the core's own slice.
    srcslot = np.full((NTILES, EPT), -1, np.int16)
    for t, nodes in enumerate(tiles_nodes):
        e = 0
        for j, nd in enumerate(nodes):
            s0, s1 = int(starts[nd]), int(starts[nd + 1])
            ne = s1 - s0
            if ne:
                q[t, e:e + ne, 0:4, j] = sea[s0:s1]
                q[t, e:e + ne, 4, j] = 1.0
                srcslot[t, e:e + ne] = perm[ssrc[s0:s1]].astype(np.int16)
                e += ne
    for k in range(NCORES):
        base = k * NC_COLS
        block = srcslot[k * T:(k + 1) * T].reshape(-1)
        holes = np.flatnonzero(block < 0)
        block[holes] = base + np.arange(holes.size) % NC_COLS
        srcslot[k * T:(k + 1) * T] = block.reshape(T, EPT)

    _, _, qh_u, ql_u = _bf16_split(q)
    qhs, qls, idxs = [], [], []
    i_arange = np.arange(T * EPT)
    for k in range(NCORES):
        def qlay(qu):
            qt = qu[k * T:(k + 1) * T]
            return np.ascontiguousarray(qt.transpose(1, 0, 2, 3)).reshape(
                128, T * QW)
        qhs.append(qlay(qh_u.reshape(NTILES, EPT, NCH, SLOTS)))
        qls.append(qlay(ql_u.reshape(NTILES, EPT, NCH, SLOTS)))
        flat = srcslot[k * T:(k + 1) * T].reshape(-1)
        ia = np.zeros((128, T * 8), np.int16)
        # the index list is read per 16-partition group by each of the 8
        # GPSIMD cores on HW -> replicate it into every group
        for g in range(8):
            ia[g * 16 + i_arange % 16, i_arange // 16] = flat
        idxs.append(ia)
    return qhs, qls, idxs, perm


def _prep_inputs(inputs):
    x = np.asarray(inputs["x"], np.float32)
    qhs, qls, idxs, perm = _pack(inputs["edge_index"], inputs["edge_attr"])

    x_pad = np.zeros((NPAD, IN_F), np.float32)
    x_pad[perm] = x
    xt = np.ascontiguousarray(x_pad.T)                      # [32, NPAD]
    _, _, xh_u, xl_u = _bf16_split(xt)
    xs_full = np.concatenate([xh_u, xl_u], axis=0)          # [64, NPAD] u16
    xsos = [
        np.ascontiguousarray(xs_full[:, k * NC_COLS:(k + 1) * NC_COLS])
        for k in range(NCORES)
    ]

    def stack2(a):                                          # [m,n] -> [2m,n]
        return np.concatenate([a, a], axis=0)

    lin0_w = np.asarray(inputs["lin0_w"], np.float32)       # [32, 64]
    _, _, w0h, w0l = _bf16_split(lin0_w)
    l0 = np.zeros((64, 128), np.uint16)
    l0[:, 0:64] = stack2(w0h)
    l0[:, 64:128] = stack2(w0l)

    nw = np.asarray(inputs["nn_w"], np.float32)
    ws = np.zeros((128, NCH * 128), np.uint16)
    for c in range(NCH):
        wc = (nw[c].reshape(H, H) if c < 4
              else np.asarray(inputs["nn_b"], np.float32).reshape(H, H))
        _, _, wch, wcl = _bf16_split(wc)
        ws[:, c * 128:c * 128 + 64] = stack2(wch)
        ws[:, c * 128 + 64:c * 128 + 128] = stack2(wcl)

    root_w = np.asarray(inputs["root_w"], np.float32)
    _, _, rh, rl = _bf16_split(root_w)
    root = np.concatenate([rh, rl], axis=0)                 # [128, 64]

    wih_t = np.ascontiguousarray(np.asarray(inputs["gru_w_ih"], np.float32).T)
    whh_t = np.ascontiguousarray(np.asarray(inputs["gru_w_hh"], np.float32).T)
    _, _, wih_h, wih_l = _bf16_split(wih_t)                 # [64, 192]
    _, _, whh_h, whh_l = _bf16_split(whh_t)
    gru = np.zeros((128, 384), np.uint16)
    gru[0:64, 0:128] = wih_h[:, 0:128]                      # Vih_rz
    gru[64:128, 0:128] = wih_l[:, 0:128]
    gru[0:64, 128:256] = whh_h[:, 0:128]                    # Vhh_rz
    gru[64:128, 128:256] = whh_l[:, 0:128]
    gru[0:64, 256:320] = wih_h[:, 128:192]                  # Vih_n
    gru[64:128, 256:320] = wih_l[:, 128:192]
    gru[0:64, 320:384] = whh_h[:, 128:192]                  # Vhh_n
    gru[64:128, 320:384] = whh_l[:, 128:192]

    b_ih = np.asarray(inputs["gru_b_ih"], np.float32)
    b_hh = np.asarray(inputs["gru_b_hh"], np.float32)
    bias_pack = np.zeros((128, 8), np.float32)
    bias_pack[0:64, 0] = np.asarray(inputs["lin0_b"], np.float32)
    bias_pack[0:64, 1] = np.asarray(inputs["conv_b"], np.float32)
    bias_pack[0:64, 2] = (b_ih + b_hh)[0:64]                # b_r
    bias_pack[64:128, 2] = (b_ih + b_hh)[64:128]            # b_z
    bias_pack[0:64, 4] = b_ih[128:192]
    bias_pack[0:64, 5] = b_hh[128:192]
    identf = np.eye(64, dtype=np.float32)
    _, ident_u = _bf16_rne(np.eye(128, dtype=np.float32))

    in_maps = []
    for k in range(NCORES):
        in_maps.append(
            {
                "qh_in": qhs[k],
                "ql_in": qls[k],
                "idx_in": idxs[k],
                "xs_in": xs_full,
                "xso_in": xsos[k],
                "l0_in": l0,
                "ws_in": ws,
                "root_in": root,
                "gru_in": gru,
                "bias_in": bias_pack,
                "ident_in": ident_u,
                "identf_in": identf,
            }
        )
    return in_maps, perm


def _assemble(results, perm):
    full = np.concatenate([results[k]["out_sl"] for k in range(NCORES)], axis=0)
    return np.ascontiguousarray(full[perm]).astype(np.float32)


def kernel(**inputs) -> np.ndarray:
    in_maps, perm = _prep_inputs(inputs)
    nc = _get_nc()
    if os.environ.get("BASS_KERNEL_SIM"):
        results = _run_sim(nc, in_maps)
    else:
        from concourse import bass_utils

        res = bass_utils.run_bass_kernel_spmd(
            nc, in_maps, core_ids=list(range(NCORES))
        )
        results = res.results
    return _assemble(results, perm)


def _run_sim(nc, in_maps):
    from concourse.bass_interp import MultiCoreSim

    sim = MultiCoreSim(nc, num_cores=NCORES, trace=False)
    for k, core in sim.cores.items():
        for name, arr in in_maps[k].items():
            core.tensor(name)[:] = arr
    sim.simulate(check_with_hw=False)
    out = []
    for k in range(NCORES):
        out.append({"out_sl": np.array(sim.cores[k].tensor("out_sl"))})
    return out


if __name__ == "__main__":
    rng = np.random.default_rng(0)
    demo = {
        "x": rng.standard_normal((N_NODES, IN_F), dtype=np.float32),
        "edge_index": rng.integers(0, N_NODES, (2, N_EDGES)).astype(np.int32),
        "edge_attr": rng.random((N_EDGES, 4), dtype=np.float32),
        "lin0_w": rng.standard_normal((IN_F, H), dtype=np.float32) * 0.1,
        "lin0_b": np.zeros(H, np.float32),
        "nn_w": rng.standard_normal((4, H * H), dtype=np.float32) * 0.05,
        "nn_b": np.zeros(H * H, np.float32),
        "root_w": rng.standard_normal((H, H), dtype=np.float32) * 0.1,
        "conv_b": np.zeros(H, np.float32),
        "gru_w_ih": rng.standard_normal((3 * H, H), dtype=np.float32) * 0.1,
        "gru_w_hh": rng.standard_normal((3 * H, H), dtype=np.float32) * 0.1,
        "gru_b_ih": np.zeros(3 * H, np.float32),
        "gru_b_hh": np.zeros(3 * H, np.float32),
    }
    out = kernel(**demo)
    print("kernel output", out.shape, out.dtype, float(np.abs(out).mean()))
